# revision 24
# baseline (speedup 1.0000x reference)
import sys, os
sys.path.insert(0, "/opt/trn_rl_repo")
os.environ.setdefault("NEURON_RT_LOG_LEVEL", "WARNING")
import numpy as np
import ml_dtypes

import concourse.bass as bass
import concourse.bacc as bacc
import concourse.mybir as mybir
import concourse.tile as tile
from concourse import masks
from concourse.bass_utils import run_bass_kernel_spmd

dt = mybir.dt
bf16 = ml_dtypes.bfloat16
NC = 8
N = 50000
NPC = N // NC
TPC = (NPC + 127) // 128
NPAD = TPC * 128
HALF = 32768
G = 4
DMA_SHARE_PM = 450   # permille of pass-1/2 edges routed through the dma machinery


def build_dma_layout(src, dst, half=HALF):
    """Edge layout for the dma_gather machinery: per-core, per-dst-tile,
    per-src-half 128-padded segments, grouped G tiles at a time.
    Returns (cfg, per_core list of dict(idx, dl))."""
    core = dst // NPC
    dstl = dst - core * NPC
    tl = dstl >> 7
    dl128 = (dstl & 127).astype(np.float32)
    h = (src >= half).astype(np.int64)

    order = np.lexsort((src, h, tl, core))
    s_src = src[order]
    s_dl = dl128[order]

    key = (core * TPC + tl) * 2 + h
    cnt = np.bincount(key, minlength=NC * TPC * 2).reshape(NC, TPC, 2)
    m = np.maximum(cnt.max(axis=0), 0)
    m = ((m + 127) // 128 * 128).astype(np.int64)          # [TPC, 2]

    segs = [[] for _ in range(TPC)]
    groups = []          # list of (h -> list of (t, pos, len))
    pos = 0
    for g0 in range(0, TPC, G):
        tls = list(range(g0, min(g0 + G, TPC)))
        gmeta = {0: [], 1: []}
        for hh in (0, 1):
            for t in tls:
                L = int(m[t, hh])
                if L:
                    gmeta[hh].append((t, pos, L))
                    segs[t].append((hh, pos, L))
                    pos += L
        groups.append(gmeta)
    TOT = pos
    assert TOT % 128 == 0

    bounds = np.zeros(NC * TPC * 2 + 1, np.int64)
    bounds[1:] = np.cumsum(cnt.reshape(-1))

    per_core = []
    for c in range(NC):
        idx_arr = np.zeros(TOT, np.int32)
        dl_arr = np.full(TOT, -1.0, np.float32)
        for t in range(TPC):
            for hh, spos, L in segs[t]:
                k = (c * TPC + t) * 2 + hh
                a, b = bounds[k], bounds[k + 1]
                n = b - a
                idx_arr[spos:spos + n] = s_src[a:b] - hh * half
                dl_arr[spos:spos + n] = s_dl[a:b]
        assert idx_arr.max(initial=0) < 32768
        idx_w = np.tile(idx_arr.astype(np.int16).reshape(TOT // 16, 16).T,
                        (8, 1)).copy()
        dl_w = dl_arr.reshape(TOT // 128, 128).T.astype(bf16).copy()
        per_core.append(dict(idx=idx_w, dl=dl_w))
    cfg = dict(TOT=TOT, segs=segs, groups=groups)
    return cfg, per_core


def build_ap_layout(src_, dst):
    """Layout for the ap_gather machinery (SBUF-table gathers): 8 slot-sets
    (one per source core), positions grouped by dst-tile group."""
    core = dst // NPC
    dstl = dst - core * NPC
    tl_all = (dstl >> 7).astype(np.int64)
    dl_all = (dstl & 127).astype(np.int64)
    q_all = src_ // NPC
    qi_all = (src_ - q_all * NPC).astype(np.int64)

    ngroups = (TPC + G - 1) // G
    streams = [[[None] * 8 for _ in range(ngroups)] for _ in range(NC)]
    for c in range(NC):
        m = core == c
        s_q, s_qi, s_tl, s_dl = q_all[m], qi_all[m], tl_all[m], dl_all[m]
        order = np.lexsort((s_qi, s_tl, s_q))
        s_q, s_qi, s_tl, s_dl = (a[order] for a in (s_q, s_qi, s_tl, s_dl))
        g_of = s_tl // G
        for g in range(ngroups):
            gm = g_of == g
            for qq in range(8):
                mm = gm & (s_q == qq)
                streams[c][g][qq] = (s_qi[mm], s_tl[mm], s_dl[mm])

    groups = []            # uniform: per group dict(t0,t1,pos0,L,blocks)
    dl_cols = [[] for _ in range(NC)]   # per-core list of [128] arrays
    idx_parts = [[] for _ in range(NC)]
    pos0 = 0
    for g in range(ngroups):
        t0, t1 = g * G, min((g + 1) * G, TPC)
        Lmax = max(len(streams[c][g][qq][0]) for c in range(NC) for qq in range(8))
        L = max(128, (Lmax + 127) // 128 * 128)
        nblk = L // 128
        for c in range(NC):
            blk = np.zeros((L, 8), np.int16)
            for qq in range(8):
                qi = streams[c][g][qq][0]
                blk[:len(qi), qq] = qi.astype(np.int16)
            idx_parts[c].append(blk)
        blocks = [[] for _ in range(nblk)]
        for b in range(nblk):
            for qq in range(8):
                tiles = set()
                for c in range(NC):
                    qt = streams[c][g][qq][1]
                    seg = qt[b * 128:min((b + 1) * 128, len(qt))]
                    tiles.update(int(t) for t in np.unique(seg))
                for tt in sorted(tiles):
                    col = len(dl_cols[0])
                    for c in range(NC):
                        qt = streams[c][g][qq][1]
                        qd = streams[c][g][qq][2]
                        dv = np.full(128, -1.0, np.float32)
                        seg_t = qt[b * 128:min((b + 1) * 128, len(qt))]
                        seg_d = qd[b * 128:min((b + 1) * 128, len(qd))]
                        sel = seg_t == tt
                        dv[:len(seg_t)][sel] = seg_d[sel]
                        dl_cols[c].append(dv)
                    blocks[b].append((col, tt, qq))
        groups.append(dict(t0=t0, t1=t1, pos0=pos0, L=L, blocks=blocks))
        pos0 += L
    TOTP = pos0
    NCOLR = len(dl_cols[0])
    NCOL = (NCOLR + 15) // 16 * 16
    per_core = []
    for c in range(NC):
        idx_all = np.concatenate(idx_parts[c], axis=0)      # [TOTP, 8]
        idx8 = np.zeros((128, TOTP // 16), np.int16)
        for qq in range(8):
            idx8[16 * qq:16 * (qq + 1), :] = idx_all[:, qq].reshape(TOTP // 16, 16).T
        dl8 = np.full((128, NCOL), -1.0, np.float32)
        for i, dv in enumerate(dl_cols[c]):
            dl8[:, i] = dv
        per_core.append(dict(idx8=idx8, dl8=dl8.astype(bf16)))
    return groups, per_core, TOTP, NCOL


def build_node_data(x, deg):
    """Per-core node tensors: tiled transposed x (dinv NOT folded; matches
    baseline), degree tables."""
    per_core = []
    for c in range(NC):
        degc = np.ones(NPAD, np.float32)
        degc[:NPC] = deg[c * NPC:(c + 1) * NPC]
        deg_pp = degc.reshape(TPC, 128).T.copy()
        deg_row = degc.reshape(1, NPAD).copy()
        xc = np.zeros((NPAD, x.shape[1]), np.float32)
        xc[:NPC] = x[c * NPC:(c + 1) * NPC]
        xtt = xc.reshape(TPC, 128, 3, 128).transpose(0, 2, 3, 1) \
                .reshape(TPC * 3 * 128, 128).astype(bf16)
        per_core.append(dict(deg_pp=deg_pp, deg_row=deg_row, xtt=xtt))
    return per_core


def split_calls(pos, L, maxc):
    out = []
    while L > 0:
        c = min(L, maxc)
        out.append((pos, c))
        pos += c
        L -= c
    return out


def build_program(full_cfg, share_cfg, ap_groups, TOTP, NCOL,
                  maxc_sh=1024, maxc128=6144, maxc256=4096):
    MAXP = max((len(blk) for g in ap_groups for blk in g["blocks"]), default=1)
    nc = bacc.Bacc("TRN2", target_bir_lowering=False, debug=False,
                   num_devices=NC)

    # ---- I/O ----
    TOTF = full_cfg["TOT"]
    TOTS = share_cfg["TOT"]
    xtt_t = nc.dram_tensor("xtt", [TPC * 3 * 128, 128], dt.bfloat16, kind="ExternalInput")
    idxf_t = nc.dram_tensor("idxf", [128, TOTF // 16], dt.int16, kind="ExternalInput")
    dlf_t = nc.dram_tensor("dlf", [128, TOTF // 128], dt.bfloat16, kind="ExternalInput")
    idxs_t = nc.dram_tensor("idxs", [128, TOTS // 16], dt.int16, kind="ExternalInput")
    dls_t = nc.dram_tensor("dls", [128, TOTS // 128], dt.bfloat16, kind="ExternalInput")
    idx8_t = nc.dram_tensor("idx8", [128, TOTP // 16], dt.int16, kind="ExternalInput")
    dl8_t = nc.dram_tensor("dl8", [128, NCOL], dt.bfloat16, kind="ExternalInput")
    degpp_t = nc.dram_tensor("deg_pp", [128, TPC], dt.float32, kind="ExternalInput")
    degrow_t = nc.dram_tensor("deg_row", [1, NPAD], dt.float32, kind="ExternalInput")
    w_t = {k: nc.dram_tensor(k, list(s), dt.bfloat16, kind="ExternalInput")
           for k, s in dict(W1=(384, 128), W2=(128, 384), W3=(384, 256),
                            W4=(256, 384), Wl=(384, 128), b1=(1, 128),
                            b2=(1, 384), b3=(1, 256), b4=(1, 384),
                            bl=(1, 128)).items()}
    out_t = nc.dram_tensor("out", [NPC, 128], dt.float32, kind="ExternalOutput")

    # ---- internal DRAM ----
    ag_in = [None, None] + [nc.dram_tensor(f"agin{i}", [NPC, 256], dt.bfloat16)
                            for i in (2, 3)]
    table = [None, None] + [nc.dram_tensor(f"table{i}", [N, 256], dt.bfloat16,
                                           addr_space="Shared") for i in (2, 3)]
    # interleaved ap-gather tables for passes 1,2
    tin = [nc.dram_tensor(f"tin{i}", [16, NPC, 8], dt.bfloat16) for i in (0, 1)]
    slf = [nc.dram_tensor(f"slf{i}", [NPC, 128], dt.bfloat16) for i in (0, 1)]
    # node-major allgathered T1/T2 tables for the dma-share of passes 1,2
    tnm = [nc.dram_tensor(f"tnm{i}", [N, 128], dt.bfloat16, addr_space="Shared")
           for i in (0, 1)]
    s2d = nc.dram_tensor("s2d", [NPC, 128], dt.bfloat16)
    tout = [nc.dram_tensor(f"tout{i}", [128, NPC, 8], dt.bfloat16,
                           addr_space="Shared") for i in (0, 1)]

    f32, bft = dt.float32, dt.bfloat16

    with tile.TileContext(nc) as tc:
        with tc.tile_pool(name="const", bufs=1) as cp:
            # ---------- constants / persistent ----------
            iota_b = cp.tile([128, 128], bft)
            with tc.tile_pool(name="scr0", bufs=1) as scr0:
                iota_i = scr0.tile([128, 128], dt.int32)
                nc.gpsimd.iota(iota_i[:], pattern=[[1, 128]], base=0,
                               channel_multiplier=0)
                nc.vector.tensor_copy(iota_b[:], iota_i[:])
            ident_b = cp.tile([128, 128], bft)
            masks.make_identity(nc, ident_b[:])
            ones_row = cp.tile([1, 128], bft)
            nc.gpsimd.memset(ones_row[:], 1.0)

            idx8_sb = cp.tile([128, TOTP // 16], dt.int16)
            nc.sync.dma_start(out=idx8_sb[:], in_=idx8_t[:, :])
            dl8_sb = cp.tile([128, NCOL], bft)
            nc.sync.dma_start(out=dl8_sb[:], in_=dl8_t[:, :])


            def wtiles(name, K, F):
                ts = []
                for k in range(K // 128):
                    w = cp.tile([128, F], bft, tag=f"{name}{k}")
                    nc.sync.dma_start(out=w[:], in_=w_t[name][k * 128:(k + 1) * 128, :])
                    ts.append(w)
                return ts
            W1sb = wtiles("W1", 384, 128)
            W2sb = wtiles("W2", 128, 384)
            W3sb = wtiles("W3", 384, 256)
            W4sb = wtiles("W4", 256, 384)
            Wlsb = wtiles("Wl", 384, 128)
            brow = {}
            for name, F in [("b1", 128), ("b2", 384), ("b3", 256), ("b4", 384), ("bl", 128)]:
                b = cp.tile([1, F], bft, tag=name)
                nc.sync.dma_start(out=b[:], in_=w_t[name][:, :])
                brow[name] = b

            deg_pp = cp.tile([128, TPC], f32)
            nc.sync.dma_start(out=deg_pp[:], in_=degpp_t[:, :])
            sq_pp = cp.tile([128, TPC], f32)
            nc.scalar.activation(sq_pp[:], deg_pp[:], mybir.ActivationFunctionType.Sqrt)
            dinv_pp = cp.tile([128, TPC], f32)
            nc.vector.reciprocal(dinv_pp[:], sq_pp[:])
            deginv_pp = cp.tile([128, TPC], f32)
            nc.vector.reciprocal(deginv_pp[:], deg_pp[:])
            sq_row = cp.tile([1, NPAD], bft)
            with tc.tile_pool(name="scr1", bufs=1) as scr1:
                deg_row = scr1.tile([1, NPAD], f32)
                nc.sync.dma_start(out=deg_row[:], in_=degrow_t[:, :])
                nc.scalar.activation(sq_row[:], deg_row[:],
                                     mybir.ActivationFunctionType.Sqrt)

            def act_leaky(out_ap, ps_ap, scale_tile, t):
                nc.scalar.activation(out_ap, ps_ap,
                                     mybir.ActivationFunctionType.Lrelu,
                                     bias=0.0, scale=scale_tile[:, t:t + 1],
                                     alpha=0.01)

            nv = lambda t: min(128, NPC - t * 128)

            def write_tin(ti, t, src_nm, ittp, trp16):  # ittp: pool for itt
                # src_nm: [128 node, 128 col] bf16 node-major tile; col p is
                # stored at DRAM (c=p//8, i, j=p%8); the gather returns col
                # (8c+j) at rhs slot (16j+c) -- pi perm folded into host data.
                n = nv(t)
                itt = ittp.tile([16, 1024], bft, tag="itt")
                itt3 = itt[:].rearrange("c (i j) -> c i j", j=8)
                src3 = src_nm[:].rearrange("p (c j) -> p c j", c=16, j=8)
                trjB = trp16.tile([16, 1024], bft, tag="trjB")
                for j in range(8):
                    nc.tensor.matmul(trjB[:, j * 128:(j + 1) * 128],
                                     lhsT=src3[:, :, j], rhs=ident_b[:],
                                     is_transpose=True)
                nc.vector.tensor_copy(
                    itt3, trjB[:].rearrange("c (j i) -> c i j", j=8))
                nc.sync.dma_start(out=tin[ti][:, t * 128:t * 128 + n, :],
                                  in_=itt3[:, :n, :])

            # ---------- phase B: dense1 -> T1 (stash + transposed table) ----------
            with tc.tile_pool(name="xp", bufs=6) as xp, \
                 tc.tile_pool(name="t1p", bufs=4) as t1p, \
                 tc.tile_pool(name="ittB", bufs=2) as ittB, \
                 tc.tile_pool(name="psB", bufs=4, space="PSUM") as psB, \
                 tc.tile_pool(name="trB", bufs=2, space="PSUM") as trB:
                for t in range(TPC):
                    xts = []
                    for k in range(3):
                        xt = xp.tile([128, 128], bft, tag="xt")
                        r0 = (t * 3 + k) * 128
                        nc.sync.dma_start(out=xt[:], in_=xtt_t[r0:r0 + 128, :])
                        xts.append(xt)
                    ps = psB.tile([128, 128], f32, tag="ps1")
                    for k in range(3):
                        nc.tensor.matmul(ps[:], lhsT=xts[k][:], rhs=W1sb[k][:],
                                         start=(k == 0), stop=(k == 2))
                    T1t = t1p.tile([128, 128], bft, tag="t1")
                    nc.vector.tensor_scalar(T1t[:], ps[:], dinv_pp[:, t:t + 1], None,
                                            mybir.AluOpType.mult)
                    T1p = t1p.tile([128, 128], bft, tag="t1p")
                    nc.vector.tensor_copy(
                        T1p[:].rearrange("p (j c) -> p j c", j=8, c=16),
                        T1t[:].rearrange("p (c j) -> p j c", c=16, j=8))
                    nc.sync.dma_start(out=slf[0][t * 128:t * 128 + nv(t), :],
                                      in_=T1p[:nv(t), :])
                    write_tin(0, t, T1t, ittB, trB)

            def allgather_ap(i):
                nc.gpsimd.collective_compute(
                    "AllGather", mybir.AluOpType.bypass,
                    replica_groups=[list(range(NC))],
                    ins=[tin[i].ap().opt()], outs=[tout[i].ap().opt()])

            def allgather_nm(i):
                nc.gpsimd.collective_compute(
                    "AllGather", mybir.AluOpType.bypass,
                    replica_groups=[list(range(NC))],
                    ins=[slf[i].ap().opt()], outs=[tnm[i].ap().opt()])

            def allgather(i):
                nc.gpsimd.collective_compute(
                    "AllGather", mybir.AluOpType.bypass,
                    replica_groups=[list(range(NC))],
                    ins=[ag_in[i].ap().opt()], outs=[table[i].ap().opt()])

            allgather_ap(0)
            allgather_nm(0)

            # ---------- hybrid aggregation passes 1 & 2 ----------
            CALL = 256
            with tc.tile_pool(name="tbp", bufs=1) as tbp, \
                 tc.tile_pool(name="gp8", bufs=2) as gp8, \
                 tc.tile_pool(name="gsp", bufs=2) as gsp, \
                 tc.tile_pool(name="slp8", bufs=3) as slp8, \
                 tc.tile_pool(name="pp8", bufs=2) as pp8, \
                 tc.tile_pool(name="t2p", bufs=4) as t2p, \
                 tc.tile_pool(name="gpd", bufs=3) as gpd, \
                 tc.tile_pool(name="ppd", bufs=3) as ppd, \
                 tc.tile_pool(name="itt1", bufs=1) as itt1, \
                 tc.tile_pool(name="idp", bufs=3) as idp, \
                 tc.tile_pool(name="agg8", bufs=4, space="PSUM") as agg8, \
                 tc.tile_pool(name="psg8", bufs=2, space="PSUM") as psg8, \
                 tc.tile_pool(name="trP", bufs=2, space="PSUM") as trP:

                def hybrid_pass(tb3, slf_t, tnm_t, binit_bias, post):
                    half0 = tnm_t[0:HALF, :]
                    half1 = tnm_t[HALF:N, :]
                    sh_groups = share_cfg["groups"]
                    sh_segs = share_cfg["segs"]
                    for gi, g in enumerate(ap_groups):
                        tiles = list(range(g["t0"], g["t1"]))
                        left = {t: 0 for t in tiles}
                        for blk in g["blocks"]:
                            for (_, tt, _) in blk:
                                left[tt] += 1
                        for t in tiles:
                            left[t] += sum(L for _, _, L in sh_segs[t]) // 128
                        pst = {}
                        for t in tiles:
                            ps = agg8.tile([128, 128], f32, tag="agg", name="agg8")[:]
                            pst[t] = ps
                            if binit_bias is not None:
                                nc.tensor.matmul(ps,
                                                 lhsT=sq_row[0:1, t * 128:(t + 1) * 128],
                                                 rhs=binit_bias[:],
                                                 start=True, stop=False)
                            sl = slp8.tile([128, 128], bft, tag="sl8", name="sl8")
                            r1 = min((t + 1) * 128, NPC)
                            if r1 - t * 128 < 128:
                                nc.vector.memzero(sl[:])
                            nc.sync.dma_start(out=sl[:r1 - t * 128, :],
                                              in_=slf_t[t * 128:r1, :])
                            nc.tensor.matmul(ps, lhsT=ident_b[:], rhs=sl[:],
                                             start=(binit_bias is None),
                                             stop=(left[t] == 0))
                        # ---- interleaved emission of both machineries ----
                        def emit_ap(cpos):
                            L = g["L"]
                            n = min(CALL, L - cpos)
                            gt = gp8.tile([128, CALL * 8], bft, tag="g8")
                            g3 = gt[:, :n * 8].rearrange("p (i d) -> p i d", d=8)
                            p0 = g["pos0"] + cpos
                            nc.gpsimd.ap_gather(
                                out_ap=g3, in_ap=tb3,
                                idxs_ap=idx8_sb[:, p0 // 16:(p0 + n) // 16],
                                channels=128, num_elems=NPC, d=8, num_idxs=n)
                            for bb in range(n // 128):
                                babs = (cpos + bb * 128) // 128
                                blk = g["blocks"][babs]
                                psg = psg8.tile([128, 1024], bft, tag="psg",
                                                name="psg")
                                for j in range(8):
                                    nc.tensor.matmul(
                                        psg[:, j * 128:(j + 1) * 128],
                                        lhsT=g3[:, bb * 128:(bb + 1) * 128, j],
                                        rhs=ident_b[:], is_transpose=True)
                                gsb = gsp.tile([128, 1024], bft, tag="gsb")
                                nc.vector.tensor_copy(
                                    gsb[:].rearrange("p (q j c) -> p q j c",
                                                     q=8, j=8, c=16),
                                    psg[:].rearrange("p (j q c) -> p q j c",
                                                     j=8, q=8, c=16))
                                if not blk:
                                    continue
                                ncol = len(blk)
                                c0 = blk[0][0]
                                P = pp8.tile([128, MAXP * 128], bft, tag="P8",
                                             name="P8")
                                P3 = P[:, :ncol * 128].rearrange(
                                    "p (c d) -> p c d", d=128)
                                nc.vector.tensor_tensor(
                                    P3,
                                    iota_b[:].unsqueeze(1).broadcast_to([128, ncol, 128]),
                                    dl8_sb[:, c0:c0 + ncol].unsqueeze(2)
                                        .broadcast_to([128, ncol, 128]),
                                    mybir.AluOpType.is_equal)
                                for k, (col, tt, qq) in enumerate(blk):
                                    left[tt] -= 1
                                    nc.tensor.matmul(pst[tt],
                                                     lhsT=P[:, k * 128:(k + 1) * 128],
                                                     rhs=gsb[:, qq * 128:(qq + 1) * 128],
                                                     start=False,
                                                     stop=(left[tt] == 0))

                        def emit_dma(job):
                            hh, cpos, clen, spans = job
                            src_ap = half0 if hh == 0 else half1
                            nch = clen // 128
                            idc = idp.tile([128, maxc_sh // 16], dt.int16,
                                           tag="idc")
                            nc.sync.dma_start(
                                out=idc[:, :clen // 16],
                                in_=idxs_t[:, cpos // 16:(cpos + clen) // 16])
                            dlc = idp.tile([128, maxc_sh // 128], bft,
                                           tag="dlc")
                            nc.sync.dma_start(
                                out=dlc[:, :nch],
                                in_=dls_t[:, cpos // 128:(cpos + clen) // 128])
                            gg = gpd.tile([128, maxc_sh], bft, tag="gd", name="gd")
                            gg3 = gg[:, :clen].rearrange("p (c e) -> p c e", e=128)
                            nc.gpsimd.dma_gather(
                                out_ap=gg3, in_ap=src_ap,
                                idxs_ap=idc[:, :clen // 16],
                                num_idxs=clen, num_idxs_reg=clen, elem_size=128,
                                single_packet=False)
                            P = ppd.tile([128, maxc_sh], bft, tag="Pd", name="Pd")
                            P3 = P[:, :clen].rearrange("p (c d) -> p c d", d=128)
                            nc.vector.tensor_tensor(
                                P3,
                                iota_b[:].unsqueeze(1).broadcast_to([128, nch, 128]),
                                dlc[:, :nch]
                                    .unsqueeze(2).broadcast_to([128, nch, 128]),
                                mybir.AluOpType.is_equal)
                            for j in range(nch):
                                epos = cpos + j * 128
                                t = next(tt for tt, p0s, Ls in spans
                                         if p0s <= epos < p0s + Ls)
                                left[t] -= 1
                                nc.tensor.matmul(pst[t],
                                                 lhsT=P[:, j * 128:(j + 1) * 128],
                                                 rhs=gg[:, j * 128:(j + 1) * 128],
                                                 start=False,
                                                 stop=(left[t] == 0))

                        ap_calls = list(range(0, g["L"], CALL))
                        gmeta = sh_groups[gi]
                        dma_calls = []
                        for hh in (0, 1):
                            spans = gmeta[hh]
                            if not spans:
                                continue
                            gpos = spans[0][1]
                            gend = spans[-1][1] + spans[-1][2]
                            for cpos, clen in split_calls(gpos, gend - gpos, maxc_sh):
                                dma_calls.append((hh, cpos, clen, spans))
                        nA, nD = len(ap_calls), len(dma_calls)
                        ai = di = 0
                        while ai < nA or di < nD:
                            if di < nD and (ai >= nA or di * nA <= ai * nD):
                                emit_dma(dma_calls[di]); di += 1
                            else:
                                emit_ap(ap_calls[ai]); ai += 1
                        for t in tiles:
                            post(t, pst[t])

                tb = tbp.tile([128, NPC * 8], bft, tag="tb")
                tb3 = tb[:].rearrange("p (n d) -> p n d", d=8)
                nc.sync.dma_start(out=tb3, in_=tout[0][:, :, :])

                def post1(t, ps):
                    T2t = t2p.tile([128, 128], bft, tag="t2")
                    act_leaky(T2t[:], ps, deginv_pp, t)
                    T2p = t2p.tile([128, 128], bft, tag="t2p")
                    nc.vector.tensor_copy(
                        T2p[:].rearrange("p (j c) -> p j c", j=8, c=16),
                        T2t[:].rearrange("p (c j) -> p j c", c=16, j=8))
                    nc.sync.dma_start(out=slf[1][t * 128:t * 128 + nv(t), :],
                                      in_=T2p[:nv(t), :])
                    write_tin(1, t, T2t, itt1, trP)
                hybrid_pass(tb3, slf[0], tnm[0], brow["b1"], post1)
                allgather_ap(1)
                allgather_nm(1)

                tb2 = tbp.tile([128, NPC * 8], bft, tag="tb")
                tb23 = tb2[:].rearrange("p (n d) -> p n d", d=8)
                nc.sync.dma_start(out=tb23, in_=tout[1][:, :, :])

                def post2(t, ps):
                    o2 = t2p.tile([128, 128], bft, tag="s2o")
                    nc.vector.tensor_copy(o2[:], ps)
                    nc.sync.dma_start(out=s2d[t * 128:t * 128 + nv(t), :],
                                      in_=o2[:nv(t), :])
                hybrid_pass(tb23, slf[1], tnm[1], None, post2)

            # ---------- phase D2: dense2 + dense3 -> T3 ----------
            with tc.tile_pool(name="hp", bufs=6) as hp, \
                 tc.tile_pool(name="t3p", bufs=4) as t3p, \
                 tc.tile_pool(name="psD", bufs=2, space="PSUM") as psD, \
                 tc.tile_pool(name="trD", bufs=2, space="PSUM") as trD:
                for t in range(TPC):
                    s2l = hp.tile([128, 128], bft, tag="s2l")
                    r1 = min((t + 1) * 128, NPC)
                    if r1 - t * 128 < 128:
                        nc.vector.memzero(s2l[:])
                    nc.sync.dma_start(out=s2l[:r1 - t * 128, :],
                                      in_=s2d[t * 128:r1, :])
                    trs = trD.tile([128, 128], bft, tag="trs")
                    nc.tensor.matmul(trs[:], lhsT=s2l[:],
                                     rhs=ident_b[:], is_transpose=True)
                    s2t = hp.tile([128, 128], bft, tag="s2t")
                    nc.vector.tensor_copy(s2t[:], trs[:])
                    ps2 = psD.tile([128, 384], f32, tag="ps2")
                    nc.tensor.matmul(ps2[:], lhsT=sq_row[0:1, t * 128:(t + 1) * 128],
                                     rhs=brow["b2"][:], start=True, stop=False)
                    nc.tensor.matmul(ps2[:], lhsT=s2t[:],
                                     rhs=W2sb[0][:], start=False, stop=True)
                    h2 = hp.tile([128, 384], bft, tag="h2")
                    act_leaky(h2[:], ps2[:], dinv_pp, t)
                    trp = trD.tile([128, 384], bft, tag="tr")
                    for k in range(3):
                        nc.tensor.matmul(trp[:, k * 128:(k + 1) * 128],
                                         lhsT=h2[:, k * 128:(k + 1) * 128],
                                         rhs=ident_b[:], is_transpose=True)
                    h2t = hp.tile([128, 384], bft, tag="h2t")
                    nc.vector.tensor_copy(h2t[:], trp[:])
                    ps3 = psD.tile([128, 256], f32, tag="ps3")
                    for k in range(3):
                        nc.tensor.matmul(ps3[:], lhsT=h2t[:, k * 128:(k + 1) * 128],
                                         rhs=W3sb[k][:], start=(k == 0), stop=(k == 2))
                    T3t = t3p.tile([128, 256], bft, tag="t3")
                    nc.vector.tensor_scalar(T3t[:], ps3[:], dinv_pp[:, t:t + 1], None,
                                            mybir.AluOpType.mult)
                    nc.sync.dma_start(out=ag_in[2][t * 128:t * 128 + nv(t), :],
                                      in_=T3t[:nv(t), :])
            allgather(2)

            # ---------- late consts for full dma-gather passes ----------
            s4nm = cp.tile([128, 2 * NPAD], bft)       # S4 raw agg [node, 256]
            idxf_sb = cp.tile([128, TOTF // 16], dt.int16)
            nc.sync.dma_start(out=idxf_sb[:], in_=idxf_t[:, :])
            dlf_sb = cp.tile([128, TOTF // 128], bft)
            nc.sync.dma_start(out=dlf_sb[:], in_=dlf_t[:, :])

            # ---------- generic dma-gather aggregation pass (E / F1) ----------
            def agg_pass(pi, F, post, binit_bias=None):
                maxc = maxc128 if F == 128 else maxc256
                tab = table[pi]
                half0 = tab[0:HALF, :]
                half1 = tab[HALF:N, :]
                with tc.tile_pool(name=f"g{pi}", bufs=3) as gp, \
                     tc.tile_pool(name=f"pp{pi}", bufs=3) as pp, \
                     tc.tile_pool(name=f"sl{pi}", bufs=3) as slp, \
                     tc.tile_pool(name=f"agg{pi}", bufs=4, space="PSUM") as ap_:
                    for gmeta in full_cfg["groups"]:
                        tiles = sorted({t for hh in (0, 1) for t, _, _ in gmeta[hh]})
                        pst = {}
                        left = {t: sum(L for _, _, L in full_cfg["segs"][t]) // 128
                                for t in tiles}
                        for t in tiles:
                            ps = ap_.tile([128, F], f32, tag="agg", name="agg")
                            pst[t] = ps
                            if binit_bias is not None:
                                nc.tensor.matmul(ps[:],
                                                 lhsT=sq_row[0:1, t * 128:(t + 1) * 128],
                                                 rhs=binit_bias[:],
                                                 start=True, stop=False)
                            sl = slp.tile([128, F], bft, tag="sl", name="sl")
                            r1 = min((t + 1) * 128, NPC)
                            if r1 - t * 128 < 128:
                                nc.vector.memzero(sl[:])
                            nc.sync.dma_start(out=sl[:r1 - t * 128, :],
                                              in_=ag_in[pi][t * 128:r1, :])
                            nc.tensor.matmul(ps[:], lhsT=ident_b[:], rhs=sl[:],
                                             start=(binit_bias is None),
                                             stop=(left[t] == 0))
                        for hh in (0, 1):
                            src_ap = half0 if hh == 0 else half1
                            spans = gmeta[hh]
                            if not spans:
                                continue
                            gpos = spans[0][1]
                            gend = spans[-1][1] + spans[-1][2]
                            for cpos, clen in split_calls(gpos, gend - gpos, maxc):
                                nch = clen // 128
                                g = gp.tile([128, nch * F], bft, tag="g", name="g")
                                g3 = g[:].rearrange("p (c e) -> p c e", e=F)
                                nc.gpsimd.dma_gather(
                                    out_ap=g3, in_ap=src_ap,
                                    idxs_ap=idxf_sb[:, cpos // 16:(cpos + clen) // 16],
                                    num_idxs=clen, num_idxs_reg=clen, elem_size=F,
                                    single_packet=False)
                                P = pp.tile([128, clen], bft, tag="P", name="P")
                                P3 = P[:].rearrange("p (c d) -> p c d", d=128)
                                nc.vector.tensor_tensor(
                                    P3,
                                    iota_b[:].unsqueeze(1).broadcast_to([128, nch, 128]),
                                    dlf_sb[:, cpos // 128:(cpos + clen) // 128]
                                        .unsqueeze(2).broadcast_to([128, nch, 128]),
                                    mybir.AluOpType.is_equal)
                                for j in range(nch):
                                    epos = cpos + j * 128
                                    t = next(tt for tt, p0, L in spans
                                             if p0 <= epos < p0 + L)
                                    left[t] -= 1
                                    nc.tensor.matmul(pst[t][:],
                                                     lhsT=P[:, j * 128:(j + 1) * 128],
                                                     rhs=g[:, j * F:(j + 1) * F],
                                                     start=False,
                                                     stop=(left[t] == 0))
                        for t in tiles:
                            post(t, pst[t])

            # ---------- pass E: agg3 -> T4 ----------
            with tc.tile_pool(name="t4p", bufs=4) as t4p:
                def post_e(t, ps):
                    T4t = t4p.tile([128, 256], bft, tag="t4")
                    act_leaky(T4t[:], ps[:], deginv_pp, t)
                    nc.sync.dma_start(out=ag_in[3][t * 128:t * 128 + nv(t), :],
                                      in_=T4t[:nv(t), :])
                agg_pass(2, 256, post_e, binit_bias=brow["b3"])
            allgather(3)

            # ---------- pass F1: agg4 -> S4 (node-major) ----------
            def post_f1(t, ps):
                nc.vector.tensor_copy(s4nm[:, t * 256:(t + 1) * 256], ps[:])
            agg_pass(3, 256, post_f1)

            # ---------- phase F2: dense4 + dense5 -> out ----------
            with tc.tile_pool(name="hp4", bufs=6) as hp4, \
                 tc.tile_pool(name="op", bufs=4) as op, \
                 tc.tile_pool(name="psF", bufs=2, space="PSUM") as psF, \
                 tc.tile_pool(name="trF", bufs=2, space="PSUM") as trF:
                for t in range(TPC):
                    s4t = hp4.tile([128, 256], bft, tag="s4t")
                    for fk in range(2):
                        trs = trF.tile([128, 128], bft, tag="trs4")
                        nc.tensor.matmul(
                            trs[:],
                            lhsT=s4nm[:, t * 256 + fk * 128:t * 256 + (fk + 1) * 128],
                            rhs=ident_b[:], is_transpose=True)
                        nc.vector.tensor_copy(s4t[:, fk * 128:(fk + 1) * 128], trs[:])
                    ps4 = psF.tile([128, 384], f32, tag="ps4")
                    nc.tensor.matmul(ps4[:], lhsT=sq_row[0:1, t * 128:(t + 1) * 128],
                                     rhs=brow["b4"][:], start=True, stop=False)
                    for fk in range(2):
                        nc.tensor.matmul(ps4[:],
                                         lhsT=s4t[:, fk * 128:(fk + 1) * 128],
                                         rhs=W4sb[fk][:], start=False, stop=(fk == 1))
                    h4 = hp4.tile([128, 384], bft, tag="h4")
                    act_leaky(h4[:], ps4[:], dinv_pp, t)
                    trp = trF.tile([128, 384], bft, tag="tr4")
                    for k in range(3):
                        nc.tensor.matmul(trp[:, k * 128:(k + 1) * 128],
                                         lhsT=h4[:, k * 128:(k + 1) * 128],
                                         rhs=ident_b[:], is_transpose=True)
                    h4t = hp4.tile([128, 384], bft, tag="h4t")
                    nc.vector.tensor_copy(h4t[:], trp[:])
                    ps5 = psF.tile([128, 128], f32, tag="ps5")
                    nc.tensor.matmul(ps5[:], lhsT=ones_row[:], rhs=brow["bl"][:],
                                     start=True, stop=False)
                    for k in range(3):
                        nc.tensor.matmul(ps5[:], lhsT=h4t[:, k * 128:(k + 1) * 128],
                                         rhs=Wlsb[k][:], start=False, stop=(k == 2))
                    ot = op.tile([128, 128], f32, tag="o")
                    nc.scalar.activation(ot[:], ps5[:], mybir.ActivationFunctionType.Relu)
                    nc.sync.dma_start(out=out_t[t * 128:t * 128 + nv(t), :],
                                      in_=ot[:nv(t), :])

    nc.compile()
    return nc


def kernel(x, edge_index, W1, b1, W2, b2, W3, b3, W4, b4, Wl, bl,
           trace=False):
    x = np.asarray(x, dtype=np.float32)
    edge_index = np.asarray(edge_index)
    src = edge_index[0].astype(np.int64)
    dst = edge_index[1].astype(np.int64)
    deg = (np.bincount(dst, minlength=N) + 1).astype(np.float32)

    hsh = (src * 2654435761 + dst * 40503) % 1000
    mdma = hsh < DMA_SHARE_PM
    full_cfg, full_core = build_dma_layout(src, dst)
    share_cfg, share_core = build_dma_layout(src[mdma], dst[mdma])
    ap_groups, ap_core, TOTP, NCOL = build_ap_layout(src[~mdma], dst[~mdma])
    node_core = build_node_data(x, deg)

    piarr = np.array([8 * (m % 16) + m // 16 for m in range(128)])
    pi2 = piarr[piarr]
    W2p = np.asarray(W2)[pi2, :]
    b1p = np.asarray(b1)[piarr]
    wshared = dict(
        W1=np.asarray(W1).astype(bf16), W2=W2p.astype(bf16),
        W3=np.asarray(W3).astype(bf16), W4=np.asarray(W4).astype(bf16),
        Wl=np.asarray(Wl).astype(bf16),
        b1=b1p.reshape(1, -1).astype(bf16),
        b2=np.asarray(b2).reshape(1, -1).astype(bf16),
        b3=np.asarray(b3).reshape(1, -1).astype(bf16),
        b4=np.asarray(b4).reshape(1, -1).astype(bf16),
        bl=np.asarray(bl).reshape(1, -1).astype(bf16),
    )

    nc = build_program(full_cfg, share_cfg, ap_groups, TOTP, NCOL)
    in_maps = []
    for c in range(NC):
        m = {"xtt": node_core[c]["xtt"],
             "idxf": full_core[c]["idx"], "dlf": full_core[c]["dl"],
             "idxs": share_core[c]["idx"], "dls": share_core[c]["dl"],
             "deg_pp": node_core[c]["deg_pp"], "deg_row": node_core[c]["deg_row"],
             "idx8": ap_core[c]["idx8"], "dl8": ap_core[c]["dl8"]}
        m.update(wshared)
        in_maps.append(m)
    res = run_bass_kernel_spmd(nc, in_maps, core_ids=list(range(NC)),
                               trace=trace)
    out = np.concatenate([res.results[c]["out"] for c in range(NC)], axis=0)
    kernel.last_exec_time_ns = res.exec_time_ns
    kernel.last_results = res
    return out


# revision 32
# speedup vs baseline: 1.1449x; 1.1449x over previous
import sys, os
sys.path.insert(0, "/opt/trn_rl_repo")
os.environ.setdefault("NEURON_RT_LOG_LEVEL", "WARNING")
import numpy as np
import ml_dtypes

import concourse.bass as bass
import concourse.bacc as bacc
import concourse.mybir as mybir
import concourse.tile as tile
from concourse import masks
from concourse.bass_utils import run_bass_kernel_spmd

dt = mybir.dt
bf16 = ml_dtypes.bfloat16
NC = 8
N = 50000
NPC = N // NC
TPC = (NPC + 127) // 128
NPAD = TPC * 128
HALF = 32768
G = 6
DMA_SHARE_PM = 450   # permille of pass-1/2 edges routed through the dma machinery


def build_dma_layout(src, dst, half=HALF):
    """Edge layout for the dma_gather machinery: per-core, per-dst-tile,
    per-src-half 128-padded segments, grouped G tiles at a time.
    Returns (cfg, per_core list of dict(idx, dl))."""
    core = dst // NPC
    dstl = dst - core * NPC
    tl = dstl >> 7
    dl128 = (dstl & 127).astype(np.float32)
    h = (src >= half).astype(np.int64)

    order = np.lexsort((src, h, tl, core))
    s_src = src[order]
    s_dl = dl128[order]

    key = (core * TPC + tl) * 2 + h
    cnt = np.bincount(key, minlength=NC * TPC * 2).reshape(NC, TPC, 2)
    m = np.maximum(cnt.max(axis=0), 0)
    m = ((m + 127) // 128 * 128).astype(np.int64)          # [TPC, 2]

    segs = [[] for _ in range(TPC)]
    groups = []          # list of (h -> list of (t, pos, len))
    pos = 0
    for g0 in range(0, TPC, G):
        tls = list(range(g0, min(g0 + G, TPC)))
        gmeta = {0: [], 1: []}
        for hh in (0, 1):
            for t in tls:
                L = int(m[t, hh])
                if L:
                    gmeta[hh].append((t, pos, L))
                    segs[t].append((hh, pos, L))
                    pos += L
        groups.append(gmeta)
    TOT = pos
    assert TOT % 128 == 0

    bounds = np.zeros(NC * TPC * 2 + 1, np.int64)
    bounds[1:] = np.cumsum(cnt.reshape(-1))

    per_core = []
    for c in range(NC):
        idx_arr = np.zeros(TOT, np.int32)
        dl_arr = np.full(TOT, -1.0, np.float32)
        for t in range(TPC):
            for hh, spos, L in segs[t]:
                k = (c * TPC + t) * 2 + hh
                a, b = bounds[k], bounds[k + 1]
                n = b - a
                idx_arr[spos:spos + n] = s_src[a:b] - hh * half
                dl_arr[spos:spos + n] = s_dl[a:b]
        assert idx_arr.max(initial=0) < 32768
        idx_w = np.tile(idx_arr.astype(np.int16).reshape(TOT // 16, 16).T,
                        (8, 1)).copy()
        dl_w = dl_arr.reshape(TOT // 128, 128).T.astype(bf16).copy()
        per_core.append(dict(idx=idx_w, dl=dl_w))
    cfg = dict(TOT=TOT, segs=segs, groups=groups)
    return cfg, per_core


def build_ap_layout(src_, dst):
    """Layout for the ap_gather machinery (SBUF-table gathers): 8 slot-sets
    (one per source core), positions grouped by dst-tile group."""
    core = dst // NPC
    dstl = dst - core * NPC
    tl_all = (dstl >> 7).astype(np.int64)
    dl_all = (dstl & 127).astype(np.int64)
    q_all = src_ // NPC
    qi_all = (src_ - q_all * NPC).astype(np.int64)

    ngroups = (TPC + G - 1) // G
    streams = [[[None] * 8 for _ in range(ngroups)] for _ in range(NC)]
    for c in range(NC):
        m = core == c
        s_q, s_qi, s_tl, s_dl = q_all[m], qi_all[m], tl_all[m], dl_all[m]
        order = np.lexsort((s_qi, s_tl, s_q))
        s_q, s_qi, s_tl, s_dl = (a[order] for a in (s_q, s_qi, s_tl, s_dl))
        g_of = s_tl // G
        for g in range(ngroups):
            gm = g_of == g
            for qq in range(8):
                mm = gm & (s_q == qq)
                streams[c][g][qq] = (s_qi[mm], s_tl[mm], s_dl[mm])

    groups = []            # uniform: per group dict(t0,t1,pos0,L,blocks)
    dl_cols = [[] for _ in range(NC)]   # per-core list of [128] arrays
    idx_parts = [[] for _ in range(NC)]
    pos0 = 0
    for g in range(ngroups):
        t0, t1 = g * G, min((g + 1) * G, TPC)
        Lmax = max(len(streams[c][g][qq][0]) for c in range(NC) for qq in range(8))
        L = max(128, (Lmax + 127) // 128 * 128)
        nblk = L // 128
        for c in range(NC):
            blk = np.zeros((L, 8), np.int16)
            for qq in range(8):
                qi = streams[c][g][qq][0]
                blk[:len(qi), qq] = qi.astype(np.int16)
            idx_parts[c].append(blk)
        blocks = [[] for _ in range(nblk)]
        for b in range(nblk):
            for qq in range(8):
                tiles = set()
                for c in range(NC):
                    qt = streams[c][g][qq][1]
                    seg = qt[b * 128:min((b + 1) * 128, len(qt))]
                    tiles.update(int(t) for t in np.unique(seg))
                for tt in sorted(tiles):
                    col = len(dl_cols[0])
                    for c in range(NC):
                        qt = streams[c][g][qq][1]
                        qd = streams[c][g][qq][2]
                        dv = np.full(128, -1.0, np.float32)
                        seg_t = qt[b * 128:min((b + 1) * 128, len(qt))]
                        seg_d = qd[b * 128:min((b + 1) * 128, len(qd))]
                        sel = seg_t == tt
                        dv[:len(seg_t)][sel] = seg_d[sel]
                        dl_cols[c].append(dv)
                    blocks[b].append((col, tt, qq))
        groups.append(dict(t0=t0, t1=t1, pos0=pos0, L=L, blocks=blocks))
        pos0 += L
    TOTP = pos0
    NCOLR = len(dl_cols[0])
    NCOL = (NCOLR + 15) // 16 * 16
    per_core = []
    for c in range(NC):
        idx_all = np.concatenate(idx_parts[c], axis=0)      # [TOTP, 8]
        idx8 = np.zeros((128, TOTP // 16), np.int16)
        for qq in range(8):
            idx8[16 * qq:16 * (qq + 1), :] = idx_all[:, qq].reshape(TOTP // 16, 16).T
        dl8 = np.full((128, NCOL), -1.0, np.float32)
        for i, dv in enumerate(dl_cols[c]):
            dl8[:, i] = dv
        per_core.append(dict(idx8=idx8, dl8=dl8.astype(bf16)))
    return groups, per_core, TOTP, NCOL


def build_node_data(x, deg):
    """Per-core node tensors: tiled transposed x (dinv NOT folded; matches
    baseline), degree tables."""
    per_core = []
    for c in range(NC):
        degc = np.ones(NPAD, np.float32)
        degc[:NPC] = deg[c * NPC:(c + 1) * NPC]
        deg_pp = degc.reshape(TPC, 128).T.copy()
        deg_row = degc.reshape(1, NPAD).copy()
        xc = np.zeros((NPAD, x.shape[1]), np.float32)
        xc[:NPC] = x[c * NPC:(c + 1) * NPC]
        xtt = xc.reshape(TPC, 128, 3, 128).transpose(0, 2, 3, 1) \
                .reshape(TPC * 3 * 128, 128).astype(bf16)
        per_core.append(dict(deg_pp=deg_pp, deg_row=deg_row, xtt=xtt))
    return per_core


def split_calls(pos, L, maxc):
    out = []
    while L > 0:
        c = min(L, maxc)
        out.append((pos, c))
        pos += c
        L -= c
    return out


def build_program(full_cfg, share_cfg, ap_groups, TOTP, NCOL,
                  maxc_sh=1024, maxc128=6144, maxc256=4096):
    MAXP = max((len(blk) for g in ap_groups for blk in g["blocks"]), default=1)
    nc = bacc.Bacc("TRN2", target_bir_lowering=False, debug=False,
                   num_devices=NC)

    # ---- I/O ----
    TOTF = full_cfg["TOT"]
    TOTS = share_cfg["TOT"]
    xtt_t = nc.dram_tensor("xtt", [TPC * 3 * 128, 128], dt.bfloat16, kind="ExternalInput")
    idxf_t = nc.dram_tensor("idxf", [128, TOTF // 16], dt.int16, kind="ExternalInput")
    dlf_t = nc.dram_tensor("dlf", [128, TOTF // 128], dt.bfloat16, kind="ExternalInput")
    idxs_t = nc.dram_tensor("idxs", [128, TOTS // 16], dt.int16, kind="ExternalInput")
    dls_t = nc.dram_tensor("dls", [128, TOTS // 128], dt.bfloat16, kind="ExternalInput")
    idx8_t = nc.dram_tensor("idx8", [128, TOTP // 16], dt.int16, kind="ExternalInput")
    dl8_t = nc.dram_tensor("dl8", [128, NCOL], dt.bfloat16, kind="ExternalInput")
    degpp_t = nc.dram_tensor("deg_pp", [128, TPC], dt.float32, kind="ExternalInput")
    degrow_t = nc.dram_tensor("deg_row", [1, NPAD], dt.float32, kind="ExternalInput")
    w_t = {k: nc.dram_tensor(k, list(s), dt.bfloat16, kind="ExternalInput")
           for k, s in dict(W1=(384, 128), W2=(128, 384), W3=(384, 256),
                            W4=(256, 384), Wl=(384, 128), b1=(1, 128),
                            b2=(1, 384), b3=(1, 256), b4=(1, 384),
                            bl=(1, 128)).items()}
    out_t = nc.dram_tensor("out", [NPC, 128], dt.float32, kind="ExternalOutput")

    # ---- internal DRAM ----
    ag_in = [None, None] + [nc.dram_tensor(f"agin{i}", [NPC, 256], dt.bfloat16)
                            for i in (2, 3)]
    table = [None, None] + [nc.dram_tensor(f"table{i}", [N, 256], dt.bfloat16,
                                           addr_space="Shared") for i in (2, 3)]
    # interleaved ap-gather tables for passes 1,2
    tin = [nc.dram_tensor(f"tin{i}", [16, NPC, 8], dt.bfloat16) for i in (0, 1)]
    slf = [nc.dram_tensor(f"slf{i}", [NPC, 128], dt.bfloat16) for i in (0, 1)]
    # node-major allgathered T1/T2 tables for the dma-share of passes 1,2
    tnm = [nc.dram_tensor(f"tnm{i}", [N, 128], dt.bfloat16, addr_space="Shared")
           for i in (0, 1)]
    s2d = nc.dram_tensor("s2d", [NPC, 128], dt.bfloat16)
    tout = [nc.dram_tensor(f"tout{i}", [128, NPC, 8], dt.bfloat16,
                           addr_space="Shared") for i in (0, 1)]

    f32, bft = dt.float32, dt.bfloat16

    with tile.TileContext(nc) as tc:
        with tc.tile_pool(name="const", bufs=1) as cp:
            # ---------- constants / persistent ----------
            iota_b = cp.tile([128, 128], bft)
            with tc.tile_pool(name="scr0", bufs=1) as scr0:
                iota_i = scr0.tile([128, 128], dt.int32)
                nc.gpsimd.iota(iota_i[:], pattern=[[1, 128]], base=0,
                               channel_multiplier=0)
                nc.vector.tensor_copy(iota_b[:], iota_i[:])
            ident_b = cp.tile([128, 128], bft)
            masks.make_identity(nc, ident_b[:])
            ones_row = cp.tile([1, 128], bft)
            nc.gpsimd.memset(ones_row[:], 1.0)

            idx8_sb = cp.tile([128, TOTP // 16], dt.int16)
            nc.sync.dma_start(out=idx8_sb[:], in_=idx8_t[:, :])
            dl8_sb = cp.tile([128, NCOL], bft)
            nc.sync.dma_start(out=dl8_sb[:], in_=dl8_t[:, :])


            def wtiles(name, K, F):
                ts = []
                for k in range(K // 128):
                    w = cp.tile([128, F], bft, tag=f"{name}{k}")
                    nc.sync.dma_start(out=w[:], in_=w_t[name][k * 128:(k + 1) * 128, :])
                    ts.append(w)
                return ts
            W1sb = wtiles("W1", 384, 128)
            W2sb = wtiles("W2", 128, 384)
            W3sb = wtiles("W3", 384, 256)
            W4sb = wtiles("W4", 256, 384)
            Wlsb = wtiles("Wl", 384, 128)
            brow = {}
            for name, F in [("b1", 128), ("b2", 384), ("b3", 256), ("b4", 384), ("bl", 128)]:
                b = cp.tile([1, F], bft, tag=name)
                nc.sync.dma_start(out=b[:], in_=w_t[name][:, :])
                brow[name] = b

            deg_pp = cp.tile([128, TPC], f32)
            nc.sync.dma_start(out=deg_pp[:], in_=degpp_t[:, :])
            sq_pp = cp.tile([128, TPC], f32)
            nc.scalar.activation(sq_pp[:], deg_pp[:], mybir.ActivationFunctionType.Sqrt)
            dinv_pp = cp.tile([128, TPC], f32)
            nc.vector.reciprocal(dinv_pp[:], sq_pp[:])
            deginv_pp = cp.tile([128, TPC], f32)
            nc.vector.reciprocal(deginv_pp[:], deg_pp[:])
            sq_row = cp.tile([1, NPAD], bft)
            with tc.tile_pool(name="scr1", bufs=1) as scr1:
                deg_row = scr1.tile([1, NPAD], f32)
                nc.sync.dma_start(out=deg_row[:], in_=degrow_t[:, :])
                nc.scalar.activation(sq_row[:], deg_row[:],
                                     mybir.ActivationFunctionType.Sqrt)

            def act_leaky(out_ap, ps_ap, scale_tile, t):
                nc.scalar.activation(out_ap, ps_ap,
                                     mybir.ActivationFunctionType.Lrelu,
                                     bias=0.0, scale=scale_tile[:, t:t + 1],
                                     alpha=0.01)

            nv = lambda t: min(128, NPC - t * 128)

            def write_tin(ti, t, src_nm, ittp, trp16, trtag="trjB"):
                # src_nm: [128 node, 128 col] bf16 node-major tile; col p is
                # stored at DRAM (c=p//8, i, j=p%8); the gather returns col
                # (8c+j) at rhs slot (16j+c) -- pi perm folded into host data.
                n = nv(t)
                itt = ittp.tile([16, 1024], bft, tag="itt")
                itt3 = itt[:].rearrange("c (i j) -> c i j", j=8)
                src3 = src_nm[:].rearrange("p (c j) -> p c j", c=16, j=8)
                trjB = trp16.tile([128, 1024], bft, tag=trtag,
                                  name=trtag)[:16, :]
                for j in range(8):
                    nc.tensor.matmul(trjB[:, j * 128:(j + 1) * 128],
                                     lhsT=src3[:, :, j], rhs=ident_b[:],
                                     is_transpose=True)
                nc.vector.tensor_copy(
                    itt3, trjB.rearrange("c (j i) -> c i j", j=8))
                nc.sync.dma_start(out=tin[ti][:, t * 128:t * 128 + n, :],
                                  in_=itt3[:, :n, :])

            # ---------- phase B: dense1 -> T1 (stash + transposed table) ----------
            with tc.tile_pool(name="xp", bufs=6) as xp, \
                 tc.tile_pool(name="t1p", bufs=4) as t1p, \
                 tc.tile_pool(name="ittB", bufs=2) as ittB, \
                 tc.tile_pool(name="psB", bufs=4, space="PSUM") as psB, \
                 tc.tile_pool(name="trB", bufs=2, space="PSUM") as trB:
                for t in range(TPC):
                    xts = []
                    for k in range(3):
                        xt = xp.tile([128, 128], bft, tag="xt")
                        r0 = (t * 3 + k) * 128
                        nc.sync.dma_start(out=xt[:], in_=xtt_t[r0:r0 + 128, :])
                        xts.append(xt)
                    ps = psB.tile([128, 128], f32, tag="ps1")
                    for k in range(3):
                        nc.tensor.matmul(ps[:], lhsT=xts[k][:], rhs=W1sb[k][:],
                                         start=(k == 0), stop=(k == 2))
                    T1t = t1p.tile([128, 128], bft, tag="t1")
                    nc.vector.tensor_scalar(T1t[:], ps[:], dinv_pp[:, t:t + 1], None,
                                            mybir.AluOpType.mult)
                    T1p = t1p.tile([128, 128], bft, tag="t1p")
                    nc.vector.tensor_copy(
                        T1p[:].rearrange("p (j c) -> p j c", j=8, c=16),
                        T1t[:].rearrange("p (c j) -> p j c", c=16, j=8))
                    nc.sync.dma_start(out=slf[0][t * 128:t * 128 + nv(t), :],
                                      in_=T1p[:nv(t), :])
                    write_tin(0, t, T1t, ittB, trB)

            def allgather_ap(i):
                nc.gpsimd.collective_compute(
                    "AllGather", mybir.AluOpType.bypass,
                    replica_groups=[list(range(NC))],
                    ins=[tin[i].ap().opt()], outs=[tout[i].ap().opt()])

            def allgather_nm(i):
                nc.gpsimd.collective_compute(
                    "AllGather", mybir.AluOpType.bypass,
                    replica_groups=[list(range(NC))],
                    ins=[slf[i].ap().opt()], outs=[tnm[i].ap().opt()])

            def allgather(i):
                nc.gpsimd.collective_compute(
                    "AllGather", mybir.AluOpType.bypass,
                    replica_groups=[list(range(NC))],
                    ins=[ag_in[i].ap().opt()], outs=[table[i].ap().opt()])

            allgather_ap(0)
            allgather_nm(0)

            # ---------- hybrid aggregation passes 1 & 2 ----------
            CALL = 256
            with tc.tile_pool(name="tbp", bufs=1) as tbp, \
                 tc.tile_pool(name="gp8", bufs=2) as gp8, \
                 tc.tile_pool(name="gsp", bufs=2) as gsp, \
                 tc.tile_pool(name="slp8", bufs=3) as slp8, \
                 tc.tile_pool(name="pp8", bufs=2) as pp8, \
                 tc.tile_pool(name="t2p", bufs=4) as t2p, \
                 tc.tile_pool(name="gpd", bufs=3) as gpd, \
                 tc.tile_pool(name="ppd", bufs=3) as ppd, \
                 tc.tile_pool(name="itt1", bufs=1) as itt1, \
                 tc.tile_pool(name="idp", bufs=3) as idp, \
                 tc.tile_pool(name="agg8", bufs=6, space="PSUM") as agg8, \
                 tc.tile_pool(name="psg8", bufs=2, space="PSUM") as psg8:

                def hybrid_pass(tb3, slf_t, tnm_t, binit_bias, post):
                    half0 = tnm_t[0:HALF, :]
                    half1 = tnm_t[HALF:N, :]
                    sh_groups = share_cfg["groups"]
                    sh_segs = share_cfg["segs"]
                    for gi, g in enumerate(ap_groups):
                        tiles = list(range(g["t0"], g["t1"]))
                        left = {t: 0 for t in tiles}
                        for blk in g["blocks"]:
                            for (_, tt, _) in blk:
                                left[tt] += 1
                        for t in tiles:
                            left[t] += sum(L for _, _, L in sh_segs[t]) // 128
                        pst = {}
                        for t in tiles:
                            ps = agg8.tile([128, 128], f32, tag="agg", name="agg8")[:]
                            pst[t] = ps
                            if binit_bias is not None:
                                nc.tensor.matmul(ps,
                                                 lhsT=sq_row[0:1, t * 128:(t + 1) * 128],
                                                 rhs=binit_bias[:],
                                                 start=True, stop=False)
                            sl = slp8.tile([128, 128], bft, tag="sl8", name="sl8")
                            r1 = min((t + 1) * 128, NPC)
                            if r1 - t * 128 < 128:
                                nc.vector.memzero(sl[:])
                            nc.sync.dma_start(out=sl[:r1 - t * 128, :],
                                              in_=slf_t[t * 128:r1, :])
                            nc.tensor.matmul(ps, lhsT=ident_b[:], rhs=sl[:],
                                             start=(binit_bias is None),
                                             stop=(left[t] == 0))
                        # ---- interleaved emission of both machineries ----
                        def emit_ap(cpos):
                            L = g["L"]
                            n = min(CALL, L - cpos)
                            gt = gp8.tile([128, CALL * 8], bft, tag="g8")
                            g3 = gt[:, :n * 8].rearrange("p (i d) -> p i d", d=8)
                            p0 = g["pos0"] + cpos
                            nc.gpsimd.ap_gather(
                                out_ap=g3, in_ap=tb3,
                                idxs_ap=idx8_sb[:, p0 // 16:(p0 + n) // 16],
                                channels=128, num_elems=NPC, d=8, num_idxs=n)
                            for bb in range(n // 128):
                                babs = (cpos + bb * 128) // 128
                                blk = g["blocks"][babs]
                                psg = psg8.tile([128, 1024], bft, tag="psg",
                                                name="psg")
                                for j in range(8):
                                    nc.tensor.matmul(
                                        psg[:, j * 128:(j + 1) * 128],
                                        lhsT=g3[:, bb * 128:(bb + 1) * 128, j],
                                        rhs=ident_b[:], is_transpose=True)
                                gsb = gsp.tile([128, 1024], bft, tag="gsb")
                                nc.vector.tensor_copy(
                                    gsb[:].rearrange("p (q j c) -> p q j c",
                                                     q=8, j=8, c=16),
                                    psg[:].rearrange("p (j q c) -> p q j c",
                                                     j=8, q=8, c=16))
                                if not blk:
                                    continue
                                ncol = len(blk)
                                c0 = blk[0][0]
                                P = pp8.tile([128, MAXP * 128], bft, tag="P8",
                                             name="P8")
                                P3 = P[:, :ncol * 128].rearrange(
                                    "p (c d) -> p c d", d=128)
                                nc.vector.tensor_tensor(
                                    P3,
                                    iota_b[:].unsqueeze(1).broadcast_to([128, ncol, 128]),
                                    dl8_sb[:, c0:c0 + ncol].unsqueeze(2)
                                        .broadcast_to([128, ncol, 128]),
                                    mybir.AluOpType.is_equal)
                                for k, (col, tt, qq) in enumerate(blk):
                                    left[tt] -= 1
                                    nc.tensor.matmul(pst[tt],
                                                     lhsT=P[:, k * 128:(k + 1) * 128],
                                                     rhs=gsb[:, qq * 128:(qq + 1) * 128],
                                                     start=False,
                                                     stop=(left[tt] == 0))

                        def emit_dma(job):
                            hh, cpos, clen, spans = job
                            src_ap = half0 if hh == 0 else half1
                            nch = clen // 128
                            idc = idp.tile([128, maxc_sh // 16], dt.int16,
                                           tag="idc")
                            nc.sync.dma_start(
                                out=idc[:, :clen // 16],
                                in_=idxs_t[:, cpos // 16:(cpos + clen) // 16])
                            dlc = idp.tile([128, maxc_sh // 128], bft,
                                           tag="dlc")
                            nc.sync.dma_start(
                                out=dlc[:, :nch],
                                in_=dls_t[:, cpos // 128:(cpos + clen) // 128])
                            gg = gpd.tile([128, maxc_sh], bft, tag="gd", name="gd")
                            gg3 = gg[:, :clen].rearrange("p (c e) -> p c e", e=128)
                            nc.gpsimd.dma_gather(
                                out_ap=gg3, in_ap=src_ap,
                                idxs_ap=idc[:, :clen // 16],
                                num_idxs=clen, num_idxs_reg=clen, elem_size=128,
                                single_packet=False)
                            P = ppd.tile([128, maxc_sh], bft, tag="Pd", name="Pd")
                            P3 = P[:, :clen].rearrange("p (c d) -> p c d", d=128)
                            nc.vector.tensor_tensor(
                                P3,
                                iota_b[:].unsqueeze(1).broadcast_to([128, nch, 128]),
                                dlc[:, :nch]
                                    .unsqueeze(2).broadcast_to([128, nch, 128]),
                                mybir.AluOpType.is_equal)
                            for j in range(nch):
                                epos = cpos + j * 128
                                t = next(tt for tt, p0s, Ls in spans
                                         if p0s <= epos < p0s + Ls)
                                left[t] -= 1
                                nc.tensor.matmul(pst[t],
                                                 lhsT=P[:, j * 128:(j + 1) * 128],
                                                 rhs=gg[:, j * 128:(j + 1) * 128],
                                                 start=False,
                                                 stop=(left[t] == 0))

                        ap_calls = list(range(0, g["L"], CALL))
                        gmeta = sh_groups[gi]
                        dma_calls = []
                        for hh in (0, 1):
                            spans = gmeta[hh]
                            if not spans:
                                continue
                            gpos = spans[0][1]
                            gend = spans[-1][1] + spans[-1][2]
                            for cpos, clen in split_calls(gpos, gend - gpos, maxc_sh):
                                dma_calls.append((hh, cpos, clen, spans))
                        nA, nD = len(ap_calls), len(dma_calls)
                        ai = di = 0
                        while ai < nA or di < nD:
                            if di < nD and (ai >= nA or di * nA <= ai * nD):
                                emit_dma(dma_calls[di]); di += 1
                            else:
                                emit_ap(ap_calls[ai]); ai += 1
                        for t in tiles:
                            post(t, pst[t])

                tb = tbp.tile([128, NPC * 8], bft, tag="tb")
                tb3 = tb[:].rearrange("p (n d) -> p n d", d=8)
                nc.sync.dma_start(out=tb3, in_=tout[0][:, :, :])

                def post1(t, ps):
                    T2t = t2p.tile([128, 128], bft, tag="t2")
                    act_leaky(T2t[:], ps, deginv_pp, t)
                    T2p = t2p.tile([128, 128], bft, tag="t2p")
                    nc.vector.tensor_copy(
                        T2p[:].rearrange("p (j c) -> p j c", j=8, c=16),
                        T2t[:].rearrange("p (c j) -> p j c", c=16, j=8))
                    nc.sync.dma_start(out=slf[1][t * 128:t * 128 + nv(t), :],
                                      in_=T2p[:nv(t), :])
                    write_tin(1, t, T2t, itt1, psg8, trtag="psg")
                hybrid_pass(tb3, slf[0], tnm[0], brow["b1"], post1)
                allgather_ap(1)
                allgather_nm(1)

                tb2 = tbp.tile([128, NPC * 8], bft, tag="tb")
                tb23 = tb2[:].rearrange("p (n d) -> p n d", d=8)
                nc.sync.dma_start(out=tb23, in_=tout[1][:, :, :])

                def post2(t, ps):
                    o2 = t2p.tile([128, 128], bft, tag="s2o")
                    nc.vector.tensor_copy(o2[:], ps)
                    nc.sync.dma_start(out=s2d[t * 128:t * 128 + nv(t), :],
                                      in_=o2[:nv(t), :])
                hybrid_pass(tb23, slf[1], tnm[1], None, post2)

            # ---------- phase D2: dense2 + dense3 -> T3 ----------
            with tc.tile_pool(name="hp", bufs=6) as hp, \
                 tc.tile_pool(name="t3p", bufs=4) as t3p, \
                 tc.tile_pool(name="psD", bufs=2, space="PSUM") as psD, \
                 tc.tile_pool(name="trD", bufs=2, space="PSUM") as trD:
                for t in range(TPC):
                    s2l = hp.tile([128, 128], bft, tag="s2l")
                    r1 = min((t + 1) * 128, NPC)
                    if r1 - t * 128 < 128:
                        nc.vector.memzero(s2l[:])
                    nc.sync.dma_start(out=s2l[:r1 - t * 128, :],
                                      in_=s2d[t * 128:r1, :])
                    trs = trD.tile([128, 128], bft, tag="trs")
                    nc.tensor.matmul(trs[:], lhsT=s2l[:],
                                     rhs=ident_b[:], is_transpose=True)
                    s2t = hp.tile([128, 128], bft, tag="s2t")
                    nc.vector.tensor_copy(s2t[:], trs[:])
                    ps2 = psD.tile([128, 384], f32, tag="ps2")
                    nc.tensor.matmul(ps2[:], lhsT=sq_row[0:1, t * 128:(t + 1) * 128],
                                     rhs=brow["b2"][:], start=True, stop=False)
                    nc.tensor.matmul(ps2[:], lhsT=s2t[:],
                                     rhs=W2sb[0][:], start=False, stop=True)
                    h2 = hp.tile([128, 384], bft, tag="h2")
                    act_leaky(h2[:], ps2[:], dinv_pp, t)
                    trp = trD.tile([128, 384], bft, tag="tr")
                    for k in range(3):
                        nc.tensor.matmul(trp[:, k * 128:(k + 1) * 128],
                                         lhsT=h2[:, k * 128:(k + 1) * 128],
                                         rhs=ident_b[:], is_transpose=True)
                    h2t = hp.tile([128, 384], bft, tag="h2t")
                    nc.vector.tensor_copy(h2t[:], trp[:])
                    ps3 = psD.tile([128, 256], f32, tag="ps3")
                    for k in range(3):
                        nc.tensor.matmul(ps3[:], lhsT=h2t[:, k * 128:(k + 1) * 128],
                                         rhs=W3sb[k][:], start=(k == 0), stop=(k == 2))
                    T3t = t3p.tile([128, 256], bft, tag="t3")
                    nc.vector.tensor_scalar(T3t[:], ps3[:], dinv_pp[:, t:t + 1], None,
                                            mybir.AluOpType.mult)
                    nc.sync.dma_start(out=ag_in[2][t * 128:t * 128 + nv(t), :],
                                      in_=T3t[:nv(t), :])
            allgather(2)

            # ---------- late consts for full dma-gather passes ----------
            s4nm = cp.tile([128, 2 * NPAD], bft)       # S4 raw agg [node, 256]
            idxf_sb = cp.tile([128, TOTF // 16], dt.int16)
            nc.sync.dma_start(out=idxf_sb[:], in_=idxf_t[:, :])
            dlf_sb = cp.tile([128, TOTF // 128], bft)
            nc.sync.dma_start(out=dlf_sb[:], in_=dlf_t[:, :])

            # ---------- generic dma-gather aggregation pass (E / F1) ----------
            def agg_pass(pi, F, post, binit_bias=None):
                maxc = maxc128 if F == 128 else maxc256
                tab = table[pi]
                half0 = tab[0:HALF, :]
                half1 = tab[HALF:N, :]
                with tc.tile_pool(name=f"g{pi}", bufs=3) as gp, \
                     tc.tile_pool(name=f"pp{pi}", bufs=3) as pp, \
                     tc.tile_pool(name=f"sl{pi}", bufs=3) as slp, \
                     tc.tile_pool(name=f"agg{pi}", bufs=6, space="PSUM") as ap_:
                    for gmeta in full_cfg["groups"]:
                        tiles = sorted({t for hh in (0, 1) for t, _, _ in gmeta[hh]})
                        pst = {}
                        left = {t: sum(L for _, _, L in full_cfg["segs"][t]) // 128
                                for t in tiles}
                        for t in tiles:
                            ps = ap_.tile([128, F], f32, tag="agg", name="agg")
                            pst[t] = ps
                            if binit_bias is not None:
                                nc.tensor.matmul(ps[:],
                                                 lhsT=sq_row[0:1, t * 128:(t + 1) * 128],
                                                 rhs=binit_bias[:],
                                                 start=True, stop=False)
                            sl = slp.tile([128, F], bft, tag="sl", name="sl")
                            r1 = min((t + 1) * 128, NPC)
                            if r1 - t * 128 < 128:
                                nc.vector.memzero(sl[:])
                            nc.sync.dma_start(out=sl[:r1 - t * 128, :],
                                              in_=ag_in[pi][t * 128:r1, :])
                            nc.tensor.matmul(ps[:], lhsT=ident_b[:], rhs=sl[:],
                                             start=(binit_bias is None),
                                             stop=(left[t] == 0))
                        for hh in (0, 1):
                            src_ap = half0 if hh == 0 else half1
                            spans = gmeta[hh]
                            if not spans:
                                continue
                            gpos = spans[0][1]
                            gend = spans[-1][1] + spans[-1][2]
                            for cpos, clen in split_calls(gpos, gend - gpos, maxc):
                                nch = clen // 128
                                g = gp.tile([128, nch * F], bft, tag="g", name="g")
                                g3 = g[:].rearrange("p (c e) -> p c e", e=F)
                                nc.gpsimd.dma_gather(
                                    out_ap=g3, in_ap=src_ap,
                                    idxs_ap=idxf_sb[:, cpos // 16:(cpos + clen) // 16],
                                    num_idxs=clen, num_idxs_reg=clen, elem_size=F,
                                    single_packet=False)
                                P = pp.tile([128, clen], bft, tag="P", name="P")
                                P3 = P[:].rearrange("p (c d) -> p c d", d=128)
                                nc.vector.tensor_tensor(
                                    P3,
                                    iota_b[:].unsqueeze(1).broadcast_to([128, nch, 128]),
                                    dlf_sb[:, cpos // 128:(cpos + clen) // 128]
                                        .unsqueeze(2).broadcast_to([128, nch, 128]),
                                    mybir.AluOpType.is_equal)
                                for j in range(nch):
                                    epos = cpos + j * 128
                                    t = next(tt for tt, p0, L in spans
                                             if p0 <= epos < p0 + L)
                                    left[t] -= 1
                                    nc.tensor.matmul(pst[t][:],
                                                     lhsT=P[:, j * 128:(j + 1) * 128],
                                                     rhs=g[:, j * F:(j + 1) * F],
                                                     start=False,
                                                     stop=(left[t] == 0))
                        for t in tiles:
                            post(t, pst[t])

            # ---------- pass E: agg3 -> T4 ----------
            with tc.tile_pool(name="t4p", bufs=4) as t4p:
                def post_e(t, ps):
                    T4t = t4p.tile([128, 256], bft, tag="t4")
                    act_leaky(T4t[:], ps[:], deginv_pp, t)
                    nc.sync.dma_start(out=ag_in[3][t * 128:t * 128 + nv(t), :],
                                      in_=T4t[:nv(t), :])
                agg_pass(2, 256, post_e, binit_bias=brow["b3"])
            allgather(3)

            # ---------- pass F1: agg4 -> S4 (node-major) ----------
            def post_f1(t, ps):
                nc.vector.tensor_copy(s4nm[:, t * 256:(t + 1) * 256], ps[:])
            agg_pass(3, 256, post_f1)

            # ---------- phase F2: dense4 + dense5 -> out ----------
            with tc.tile_pool(name="hp4", bufs=6) as hp4, \
                 tc.tile_pool(name="op", bufs=4) as op, \
                 tc.tile_pool(name="psF", bufs=2, space="PSUM") as psF, \
                 tc.tile_pool(name="trF", bufs=2, space="PSUM") as trF:
                for t in range(TPC):
                    s4t = hp4.tile([128, 256], bft, tag="s4t")
                    for fk in range(2):
                        trs = trF.tile([128, 128], bft, tag="trs4")
                        nc.tensor.matmul(
                            trs[:],
                            lhsT=s4nm[:, t * 256 + fk * 128:t * 256 + (fk + 1) * 128],
                            rhs=ident_b[:], is_transpose=True)
                        nc.vector.tensor_copy(s4t[:, fk * 128:(fk + 1) * 128], trs[:])
                    ps4 = psF.tile([128, 384], f32, tag="ps4")
                    nc.tensor.matmul(ps4[:], lhsT=sq_row[0:1, t * 128:(t + 1) * 128],
                                     rhs=brow["b4"][:], start=True, stop=False)
                    for fk in range(2):
                        nc.tensor.matmul(ps4[:],
                                         lhsT=s4t[:, fk * 128:(fk + 1) * 128],
                                         rhs=W4sb[fk][:], start=False, stop=(fk == 1))
                    h4 = hp4.tile([128, 384], bft, tag="h4")
                    act_leaky(h4[:], ps4[:], dinv_pp, t)
                    trp = trF.tile([128, 384], bft, tag="tr4")
                    for k in range(3):
                        nc.tensor.matmul(trp[:, k * 128:(k + 1) * 128],
                                         lhsT=h4[:, k * 128:(k + 1) * 128],
                                         rhs=ident_b[:], is_transpose=True)
                    h4t = hp4.tile([128, 384], bft, tag="h4t")
                    nc.vector.tensor_copy(h4t[:], trp[:])
                    ps5 = psF.tile([128, 128], f32, tag="ps5")
                    nc.tensor.matmul(ps5[:], lhsT=ones_row[:], rhs=brow["bl"][:],
                                     start=True, stop=False)
                    for k in range(3):
                        nc.tensor.matmul(ps5[:], lhsT=h4t[:, k * 128:(k + 1) * 128],
                                         rhs=Wlsb[k][:], start=False, stop=(k == 2))
                    ot = op.tile([128, 128], f32, tag="o")
                    nc.scalar.activation(ot[:], ps5[:], mybir.ActivationFunctionType.Relu)
                    nc.sync.dma_start(out=out_t[t * 128:t * 128 + nv(t), :],
                                      in_=ot[:nv(t), :])

    nc.compile()
    return nc


def kernel(x, edge_index, W1, b1, W2, b2, W3, b3, W4, b4, Wl, bl,
           trace=False):
    x = np.asarray(x, dtype=np.float32)
    edge_index = np.asarray(edge_index)
    src = edge_index[0].astype(np.int64)
    dst = edge_index[1].astype(np.int64)
    deg = (np.bincount(dst, minlength=N) + 1).astype(np.float32)

    hsh = (src * 2654435761 + dst * 40503) % 1000
    mdma = hsh < DMA_SHARE_PM
    full_cfg, full_core = build_dma_layout(src, dst)
    share_cfg, share_core = build_dma_layout(src[mdma], dst[mdma])
    ap_groups, ap_core, TOTP, NCOL = build_ap_layout(src[~mdma], dst[~mdma])
    node_core = build_node_data(x, deg)

    piarr = np.array([8 * (m % 16) + m // 16 for m in range(128)])
    pi2 = piarr[piarr]
    W2p = np.asarray(W2)[pi2, :]
    b1p = np.asarray(b1)[piarr]
    wshared = dict(
        W1=np.asarray(W1).astype(bf16), W2=W2p.astype(bf16),
        W3=np.asarray(W3).astype(bf16), W4=np.asarray(W4).astype(bf16),
        Wl=np.asarray(Wl).astype(bf16),
        b1=b1p.reshape(1, -1).astype(bf16),
        b2=np.asarray(b2).reshape(1, -1).astype(bf16),
        b3=np.asarray(b3).reshape(1, -1).astype(bf16),
        b4=np.asarray(b4).reshape(1, -1).astype(bf16),
        bl=np.asarray(bl).reshape(1, -1).astype(bf16),
    )

    nc = build_program(full_cfg, share_cfg, ap_groups, TOTP, NCOL)
    in_maps = []
    for c in range(NC):
        m = {"xtt": node_core[c]["xtt"],
             "idxf": full_core[c]["idx"], "dlf": full_core[c]["dl"],
             "idxs": share_core[c]["idx"], "dls": share_core[c]["dl"],
             "deg_pp": node_core[c]["deg_pp"], "deg_row": node_core[c]["deg_row"],
             "idx8": ap_core[c]["idx8"], "dl8": ap_core[c]["dl8"]}
        m.update(wshared)
        in_maps.append(m)
    res = run_bass_kernel_spmd(nc, in_maps, core_ids=list(range(NC)),
                               trace=trace)
    out = np.concatenate([res.results[c]["out"] for c in range(NC)], axis=0)
    kernel.last_exec_time_ns = res.exec_time_ns
    kernel.last_results = res
    return out


# revision 37
# speedup vs baseline: 1.4873x; 1.2991x over previous
import sys, os
sys.path.insert(0, "/opt/trn_rl_repo")
os.environ.setdefault("NEURON_RT_LOG_LEVEL", "WARNING")
import numpy as np
import ml_dtypes

import concourse.bass as bass
import concourse.bacc as bacc
import concourse.mybir as mybir
import concourse.tile as tile
from concourse import masks
from concourse.bass_utils import run_bass_kernel_spmd

dt = mybir.dt
bf16 = ml_dtypes.bfloat16
NC = 8
N = 50000
NPC = N // NC
TPC = (NPC + 127) // 128
NPAD = TPC * 128
HALF = 32768
G = 6
DMA_SHARE_PM = 0   # permille of pass-1/2 edges routed through the dma machinery


def build_dma_layout(src, dst, half=HALF):
    """Edge layout for the dma_gather machinery: per-core, per-dst-tile,
    per-src-half 128-padded segments, grouped G tiles at a time.
    Returns (cfg, per_core list of dict(idx, dl))."""
    core = dst // NPC
    dstl = dst - core * NPC
    tl = dstl >> 7
    dl128 = (dstl & 127).astype(np.float32)
    h = (src >= half).astype(np.int64)

    order = np.lexsort((src, h, tl, core))
    s_src = src[order]
    s_dl = dl128[order]

    key = (core * TPC + tl) * 2 + h
    cnt = np.bincount(key, minlength=NC * TPC * 2).reshape(NC, TPC, 2)
    m = np.maximum(cnt.max(axis=0), 0)
    m = ((m + 127) // 128 * 128).astype(np.int64)          # [TPC, 2]

    segs = [[] for _ in range(TPC)]
    groups = []          # list of (h -> list of (t, pos, len))
    pos = 0
    for g0 in range(0, TPC, G):
        tls = list(range(g0, min(g0 + G, TPC)))
        gmeta = {0: [], 1: []}
        for hh in (0, 1):
            for t in tls:
                L = int(m[t, hh])
                if L:
                    gmeta[hh].append((t, pos, L))
                    segs[t].append((hh, pos, L))
                    pos += L
        groups.append(gmeta)
    TOT = pos
    assert TOT % 128 == 0

    bounds = np.zeros(NC * TPC * 2 + 1, np.int64)
    bounds[1:] = np.cumsum(cnt.reshape(-1))

    per_core = []
    for c in range(NC):
        idx_arr = np.zeros(TOT, np.int32)
        dl_arr = np.full(TOT, -1.0, np.float32)
        for t in range(TPC):
            for hh, spos, L in segs[t]:
                k = (c * TPC + t) * 2 + hh
                a, b = bounds[k], bounds[k + 1]
                n = b - a
                idx_arr[spos:spos + n] = s_src[a:b] - hh * half
                dl_arr[spos:spos + n] = s_dl[a:b]
        assert idx_arr.max(initial=0) < 32768
        idx_w = np.tile(idx_arr.astype(np.int16).reshape(TOT // 16, 16).T,
                        (8, 1)).copy()
        dl_w = dl_arr.reshape(TOT // 128, 128).T.astype(bf16).copy()
        per_core.append(dict(idx=idx_w, dl=dl_w))
    if TOT == 0:  # keep tensor shapes non-degenerate
        TOT = 128
        per_core = [dict(idx=np.zeros((128, 8), np.int16),
                         dl=np.full((128, 1), -1.0, np.float32).astype(bf16))
                    for _ in range(NC)]
    cfg = dict(TOT=TOT, segs=segs, groups=groups)
    return cfg, per_core


def build_ap_layout(src_, dst):
    """Layout for the ap_gather machinery (SBUF-table gathers): 8 slot-sets
    (one per source core), positions grouped by dst-tile group."""
    core = dst // NPC
    dstl = dst - core * NPC
    tl_all = (dstl >> 7).astype(np.int64)
    dl_all = (dstl & 127).astype(np.int64)
    q_all = src_ // NPC
    qi_all = (src_ - q_all * NPC).astype(np.int64)

    ngroups = (TPC + G - 1) // G
    streams = [[[None] * 8 for _ in range(ngroups)] for _ in range(NC)]
    for c in range(NC):
        m = core == c
        s_q, s_qi, s_tl, s_dl = q_all[m], qi_all[m], tl_all[m], dl_all[m]
        order = np.lexsort((s_qi, s_tl, s_q))
        s_q, s_qi, s_tl, s_dl = (a[order] for a in (s_q, s_qi, s_tl, s_dl))
        g_of = s_tl // G
        for g in range(ngroups):
            gm = g_of == g
            for qq in range(8):
                mm = gm & (s_q == qq)
                streams[c][g][qq] = (s_qi[mm], s_tl[mm], s_dl[mm])

    groups = []            # uniform: per group dict(t0,t1,pos0,L,blocks)
    dl_cols = [[] for _ in range(NC)]   # per-core list of [128] arrays
    idx_parts = [[] for _ in range(NC)]
    pos0 = 0
    for g in range(ngroups):
        t0, t1 = g * G, min((g + 1) * G, TPC)
        Lmax = max(len(streams[c][g][qq][0]) for c in range(NC) for qq in range(8))
        L = max(128, (Lmax + 127) // 128 * 128)
        nblk = L // 128
        for c in range(NC):
            blk = np.zeros((L, 8), np.int16)
            for qq in range(8):
                qi = streams[c][g][qq][0]
                blk[:len(qi), qq] = qi.astype(np.int16)
            idx_parts[c].append(blk)
        blocks = [[] for _ in range(nblk)]
        for b in range(nblk):
            for qq in range(8):
                tiles = set()
                for c in range(NC):
                    qt = streams[c][g][qq][1]
                    seg = qt[b * 128:min((b + 1) * 128, len(qt))]
                    tiles.update(int(t) for t in np.unique(seg))
                for tt in sorted(tiles):
                    col = len(dl_cols[0])
                    for c in range(NC):
                        qt = streams[c][g][qq][1]
                        qd = streams[c][g][qq][2]
                        dv = np.full(128, -1.0, np.float32)
                        seg_t = qt[b * 128:min((b + 1) * 128, len(qt))]
                        seg_d = qd[b * 128:min((b + 1) * 128, len(qd))]
                        sel = seg_t == tt
                        dv[:len(seg_t)][sel] = seg_d[sel]
                        dl_cols[c].append(dv)
                    blocks[b].append((col, tt, qq))
        groups.append(dict(t0=t0, t1=t1, pos0=pos0, L=L, blocks=blocks))
        pos0 += L
    TOTP = pos0
    NCOLR = len(dl_cols[0])
    NCOL = (NCOLR + 15) // 16 * 16
    per_core = []
    for c in range(NC):
        idx_all = np.concatenate(idx_parts[c], axis=0)      # [TOTP, 8]
        idx8 = np.zeros((128, TOTP // 16), np.int16)
        for qq in range(8):
            idx8[16 * qq:16 * (qq + 1), :] = idx_all[:, qq].reshape(TOTP // 16, 16).T
        dl8 = np.full((128, NCOL), -1.0, np.float32)
        for i, dv in enumerate(dl_cols[c]):
            dl8[:, i] = dv
        per_core.append(dict(idx8=idx8, dl8=dl8.astype(bf16)))
    return groups, per_core, TOTP, NCOL


def build_node_data(x, deg):
    """Per-core node tensors: tiled transposed x (dinv NOT folded; matches
    baseline), degree tables."""
    per_core = []
    for c in range(NC):
        degc = np.ones(NPAD, np.float32)
        degc[:NPC] = deg[c * NPC:(c + 1) * NPC]
        deg_pp = degc.reshape(TPC, 128).T.copy()
        deg_row = degc.reshape(1, NPAD).copy()
        xc = np.zeros((NPAD, x.shape[1]), np.float32)
        xc[:NPC] = x[c * NPC:(c + 1) * NPC]
        xtt = xc.reshape(TPC, 128, 3, 128).transpose(0, 2, 3, 1) \
                .reshape(TPC * 3 * 128, 128).astype(bf16)
        per_core.append(dict(deg_pp=deg_pp, deg_row=deg_row, xtt=xtt))
    return per_core


def split_calls(pos, L, maxc):
    out = []
    while L > 0:
        c = min(L, maxc)
        out.append((pos, c))
        pos += c
        L -= c
    return out


def build_program(full_cfg, share_cfg, ap_groups, TOTP, NCOL,
                  maxc_sh=1024, maxc128=6144, maxc256=4096):
    MAXP = max((len(blk) for g in ap_groups for blk in g["blocks"]), default=1)
    nc = bacc.Bacc("TRN2", target_bir_lowering=False, debug=False,
                   num_devices=NC)

    # ---- I/O ----
    TOTF = full_cfg["TOT"]
    TOTS = share_cfg["TOT"]
    xtt_t = nc.dram_tensor("xtt", [TPC * 3 * 128, 128], dt.bfloat16, kind="ExternalInput")
    idxf_t = nc.dram_tensor("idxf", [128, TOTF // 16], dt.int16, kind="ExternalInput")
    dlf_t = nc.dram_tensor("dlf", [128, TOTF // 128], dt.bfloat16, kind="ExternalInput")
    idxs_t = nc.dram_tensor("idxs", [128, TOTS // 16], dt.int16, kind="ExternalInput")
    dls_t = nc.dram_tensor("dls", [128, TOTS // 128], dt.bfloat16, kind="ExternalInput")
    idx8_t = nc.dram_tensor("idx8", [128, TOTP // 16], dt.int16, kind="ExternalInput")
    dl8_t = nc.dram_tensor("dl8", [128, NCOL], dt.bfloat16, kind="ExternalInput")
    degpp_t = nc.dram_tensor("deg_pp", [128, TPC], dt.float32, kind="ExternalInput")
    degrow_t = nc.dram_tensor("deg_row", [1, NPAD], dt.float32, kind="ExternalInput")
    w_t = {k: nc.dram_tensor(k, list(s), dt.bfloat16, kind="ExternalInput")
           for k, s in dict(W1=(384, 128), W2=(128, 384), W3=(384, 256),
                            W4=(256, 384), Wl=(384, 128), b1=(1, 128),
                            b2=(1, 384), b3=(1, 256), b4=(1, 384),
                            bl=(1, 128)).items()}
    out_t = nc.dram_tensor("out", [NPC, 128], dt.float32, kind="ExternalOutput")

    # ---- internal DRAM ----
    ag_in = [None, None] + [nc.dram_tensor(f"agin{i}", [NPC, 256], dt.bfloat16)
                            for i in (2, 3)]
    table = [None, None] + [nc.dram_tensor(f"table{i}", [N, 256], dt.bfloat16,
                                           addr_space="Shared") for i in (2, 3)]
    # interleaved ap-gather tables for passes 1,2
    tin = [nc.dram_tensor(f"tin{i}", [16, NPC, 8], dt.bfloat16) for i in (0, 1)]
    slf = [nc.dram_tensor(f"slf{i}", [NPC, 128], dt.bfloat16) for i in (0, 1)]
    # node-major allgathered T1/T2 tables for the dma-share of passes 1,2
    tnm = [nc.dram_tensor(f"tnm{i}", [N, 128], dt.bfloat16, addr_space="Shared")
           for i in (0, 1)]
    s2d = nc.dram_tensor("s2d", [NPC, 128], dt.bfloat16)
    tout = [nc.dram_tensor(f"tout{i}", [128, NPC, 8], dt.bfloat16,
                           addr_space="Shared") for i in (0, 1)]

    f32, bft = dt.float32, dt.bfloat16

    with tile.TileContext(nc) as tc:
        with tc.tile_pool(name="const", bufs=1) as cp:
            # ---------- constants / persistent ----------
            iota_b = cp.tile([128, 128], bft)
            with tc.tile_pool(name="scr0", bufs=1) as scr0:
                iota_i = scr0.tile([128, 128], dt.int32)
                nc.gpsimd.iota(iota_i[:], pattern=[[1, 128]], base=0,
                               channel_multiplier=0)
                nc.vector.tensor_copy(iota_b[:], iota_i[:])
            ident_b = cp.tile([128, 128], bft)
            masks.make_identity(nc, ident_b[:])
            ones_row = cp.tile([1, 128], bft)
            nc.gpsimd.memset(ones_row[:], 1.0)

            idx8_sb = cp.tile([128, TOTP // 16], dt.int16)
            nc.sync.dma_start(out=idx8_sb[:], in_=idx8_t[:, :])
            dl8_sb = cp.tile([128, NCOL], bft)
            nc.sync.dma_start(out=dl8_sb[:], in_=dl8_t[:, :])


            def wtiles(name, K, F):
                ts = []
                for k in range(K // 128):
                    w = cp.tile([128, F], bft, tag=f"{name}{k}")
                    nc.sync.dma_start(out=w[:], in_=w_t[name][k * 128:(k + 1) * 128, :])
                    ts.append(w)
                return ts
            W1sb = wtiles("W1", 384, 128)
            W2sb = wtiles("W2", 128, 384)
            W3sb = wtiles("W3", 384, 256)
            W4sb = wtiles("W4", 256, 384)
            Wlsb = wtiles("Wl", 384, 128)
            brow = {}
            for name, F in [("b1", 128), ("b2", 384), ("b3", 256), ("b4", 384), ("bl", 128)]:
                b = cp.tile([1, F], bft, tag=name)
                nc.sync.dma_start(out=b[:], in_=w_t[name][:, :])
                brow[name] = b

            deg_pp = cp.tile([128, TPC], f32)
            nc.sync.dma_start(out=deg_pp[:], in_=degpp_t[:, :])
            sq_pp = cp.tile([128, TPC], f32)
            nc.scalar.activation(sq_pp[:], deg_pp[:], mybir.ActivationFunctionType.Sqrt)
            dinv_pp = cp.tile([128, TPC], f32)
            nc.vector.reciprocal(dinv_pp[:], sq_pp[:])
            deginv_pp = cp.tile([128, TPC], f32)
            nc.vector.reciprocal(deginv_pp[:], deg_pp[:])
            sq_row = cp.tile([1, NPAD], bft)
            with tc.tile_pool(name="scr1", bufs=1) as scr1:
                deg_row = scr1.tile([1, NPAD], f32)
                nc.sync.dma_start(out=deg_row[:], in_=degrow_t[:, :])
                nc.scalar.activation(sq_row[:], deg_row[:],
                                     mybir.ActivationFunctionType.Sqrt)

            def act_leaky(out_ap, ps_ap, scale_tile, t):
                nc.scalar.activation(out_ap, ps_ap,
                                     mybir.ActivationFunctionType.Lrelu,
                                     bias=0.0, scale=scale_tile[:, t:t + 1],
                                     alpha=0.01)

            nv = lambda t: min(128, NPC - t * 128)

            def write_tin(ti, t, src_nm, ittp, trp16, trtag="trjB"):
                # src_nm: [128 node, 128 col] bf16 node-major tile; col p is
                # stored at DRAM (c=p//8, i, j=p%8); the gather returns col
                # (8c+j) at rhs slot (16j+c) -- pi perm folded into host data.
                n = nv(t)
                itt = ittp.tile([16, 1024], bft, tag="itt")
                itt3 = itt[:].rearrange("c (i j) -> c i j", j=8)
                src3 = src_nm[:].rearrange("p (c j) -> p c j", c=16, j=8)
                trjB = trp16.tile([128, 1024], bft, tag=trtag,
                                  name=trtag)[:16, :]
                for j in range(8):
                    nc.tensor.matmul(trjB[:, j * 128:(j + 1) * 128],
                                     lhsT=src3[:, :, j], rhs=ident_b[:],
                                     is_transpose=True)
                nc.vector.tensor_copy(
                    itt3, trjB.rearrange("c (j i) -> c i j", j=8))
                nc.sync.dma_start(out=tin[ti][:, t * 128:t * 128 + n, :],
                                  in_=itt3[:, :n, :])

            # ---------- phase B: dense1 -> T1 (stash + transposed table) ----------
            with tc.tile_pool(name="xp", bufs=6) as xp, \
                 tc.tile_pool(name="t1p", bufs=4) as t1p, \
                 tc.tile_pool(name="ittB", bufs=2) as ittB, \
                 tc.tile_pool(name="psB", bufs=4, space="PSUM") as psB, \
                 tc.tile_pool(name="trB", bufs=2, space="PSUM") as trB:
                for t in range(TPC):
                    xts = []
                    for k in range(3):
                        xt = xp.tile([128, 128], bft, tag="xt")
                        r0 = (t * 3 + k) * 128
                        nc.sync.dma_start(out=xt[:], in_=xtt_t[r0:r0 + 128, :])
                        xts.append(xt)
                    ps = psB.tile([128, 128], f32, tag="ps1")
                    for k in range(3):
                        nc.tensor.matmul(ps[:], lhsT=xts[k][:], rhs=W1sb[k][:],
                                         start=(k == 0), stop=(k == 2))
                    T1t = t1p.tile([128, 128], bft, tag="t1")
                    nc.vector.tensor_scalar(T1t[:], ps[:], dinv_pp[:, t:t + 1], None,
                                            mybir.AluOpType.mult)
                    T1p = t1p.tile([128, 128], bft, tag="t1p")
                    nc.vector.tensor_copy(
                        T1p[:].rearrange("p (j c) -> p j c", j=8, c=16),
                        T1t[:].rearrange("p (c j) -> p j c", c=16, j=8))
                    nc.sync.dma_start(out=slf[0][t * 128:t * 128 + nv(t), :],
                                      in_=T1p[:nv(t), :])
                    write_tin(0, t, T1t, ittB, trB)

            def allgather_ap(i):
                nc.gpsimd.collective_compute(
                    "AllGather", mybir.AluOpType.bypass,
                    replica_groups=[list(range(NC))],
                    ins=[tin[i].ap().opt()], outs=[tout[i].ap().opt()])

            def allgather_nm(i):
                nc.gpsimd.collective_compute(
                    "AllGather", mybir.AluOpType.bypass,
                    replica_groups=[list(range(NC))],
                    ins=[slf[i].ap().opt()], outs=[tnm[i].ap().opt()])

            def allgather(i):
                nc.gpsimd.collective_compute(
                    "AllGather", mybir.AluOpType.bypass,
                    replica_groups=[list(range(NC))],
                    ins=[ag_in[i].ap().opt()], outs=[table[i].ap().opt()])

            allgather_ap(0)
            if DMA_SHARE_PM:
                allgather_nm(0)

            # ---------- hybrid aggregation passes 1 & 2 ----------
            CALL = 256
            with tc.tile_pool(name="tbp", bufs=1) as tbp, \
                 tc.tile_pool(name="gp8", bufs=2) as gp8, \
                 tc.tile_pool(name="gsp", bufs=2) as gsp, \
                 tc.tile_pool(name="slp8", bufs=3) as slp8, \
                 tc.tile_pool(name="pp8", bufs=2) as pp8, \
                 tc.tile_pool(name="t2p", bufs=4) as t2p, \
                 tc.tile_pool(name="gpd", bufs=3) as gpd, \
                 tc.tile_pool(name="ppd", bufs=3) as ppd, \
                 tc.tile_pool(name="itt1", bufs=1) as itt1, \
                 tc.tile_pool(name="idp", bufs=3) as idp, \
                 tc.tile_pool(name="agg8", bufs=6, space="PSUM") as agg8, \
                 tc.tile_pool(name="psg8", bufs=2, space="PSUM") as psg8:

                def hybrid_pass(tb3, slf_t, tnm_t, binit_bias, post):
                    half0 = tnm_t[0:HALF, :]
                    half1 = tnm_t[HALF:N, :]
                    sh_groups = share_cfg["groups"]
                    sh_segs = share_cfg["segs"]
                    for gi, g in enumerate(ap_groups):
                        tiles = list(range(g["t0"], g["t1"]))
                        left = {t: 0 for t in tiles}
                        for blk in g["blocks"]:
                            for (_, tt, _) in blk:
                                left[tt] += 1
                        for t in tiles:
                            left[t] += sum(L for _, _, L in sh_segs[t]) // 128
                        pst = {}
                        for t in tiles:
                            ps = agg8.tile([128, 128], f32, tag="agg", name="agg8")[:]
                            pst[t] = ps
                            if binit_bias is not None:
                                nc.tensor.matmul(ps,
                                                 lhsT=sq_row[0:1, t * 128:(t + 1) * 128],
                                                 rhs=binit_bias[:],
                                                 start=True, stop=False)
                            sl = slp8.tile([128, 128], bft, tag="sl8", name="sl8")
                            r1 = min((t + 1) * 128, NPC)
                            if r1 - t * 128 < 128:
                                nc.vector.memzero(sl[:])
                            nc.sync.dma_start(out=sl[:r1 - t * 128, :],
                                              in_=slf_t[t * 128:r1, :])
                            nc.tensor.matmul(ps, lhsT=ident_b[:], rhs=sl[:],
                                             start=(binit_bias is None),
                                             stop=(left[t] == 0))
                        # ---- interleaved emission of both machineries ----
                        def emit_ap(cpos):
                            L = g["L"]
                            n = min(CALL, L - cpos)
                            gt = gp8.tile([128, CALL * 8], bft, tag="g8")
                            g3 = gt[:, :n * 8].rearrange("p (i d) -> p i d", d=8)
                            p0 = g["pos0"] + cpos
                            nc.gpsimd.ap_gather(
                                out_ap=g3, in_ap=tb3,
                                idxs_ap=idx8_sb[:, p0 // 16:(p0 + n) // 16],
                                channels=128, num_elems=NPC, d=8, num_idxs=n)
                            for bb in range(n // 128):
                                babs = (cpos + bb * 128) // 128
                                blk = g["blocks"][babs]
                                psg = psg8.tile([128, 1024], bft, tag="psg",
                                                name="psg")
                                for j in range(8):
                                    nc.tensor.matmul(
                                        psg[:, j * 128:(j + 1) * 128],
                                        lhsT=g3[:, bb * 128:(bb + 1) * 128, j],
                                        rhs=ident_b[:], is_transpose=True)
                                gsb = gsp.tile([128, 1024], bft, tag="gsb")
                                nc.vector.tensor_copy(
                                    gsb[:].rearrange("p (q j c) -> p q j c",
                                                     q=8, j=8, c=16),
                                    psg[:].rearrange("p (j q c) -> p q j c",
                                                     j=8, q=8, c=16))
                                if not blk:
                                    continue
                                ncol = len(blk)
                                c0 = blk[0][0]
                                P = pp8.tile([128, MAXP * 128], bft, tag="P8",
                                             name="P8")
                                P3 = P[:, :ncol * 128].rearrange(
                                    "p (c d) -> p c d", d=128)
                                nc.vector.tensor_tensor(
                                    P3,
                                    iota_b[:].unsqueeze(1).broadcast_to([128, ncol, 128]),
                                    dl8_sb[:, c0:c0 + ncol].unsqueeze(2)
                                        .broadcast_to([128, ncol, 128]),
                                    mybir.AluOpType.is_equal)
                                for k, (col, tt, qq) in enumerate(blk):
                                    left[tt] -= 1
                                    nc.tensor.matmul(pst[tt],
                                                     lhsT=P[:, k * 128:(k + 1) * 128],
                                                     rhs=gsb[:, qq * 128:(qq + 1) * 128],
                                                     start=False,
                                                     stop=(left[tt] == 0))

                        def emit_dma(job):
                            hh, cpos, clen, spans = job
                            src_ap = half0 if hh == 0 else half1
                            nch = clen // 128
                            idc = idp.tile([128, maxc_sh // 16], dt.int16,
                                           tag="idc")
                            nc.sync.dma_start(
                                out=idc[:, :clen // 16],
                                in_=idxs_t[:, cpos // 16:(cpos + clen) // 16])
                            dlc = idp.tile([128, maxc_sh // 128], bft,
                                           tag="dlc")
                            nc.sync.dma_start(
                                out=dlc[:, :nch],
                                in_=dls_t[:, cpos // 128:(cpos + clen) // 128])
                            gg = gpd.tile([128, maxc_sh], bft, tag="gd", name="gd")
                            gg3 = gg[:, :clen].rearrange("p (c e) -> p c e", e=128)
                            nc.gpsimd.dma_gather(
                                out_ap=gg3, in_ap=src_ap,
                                idxs_ap=idc[:, :clen // 16],
                                num_idxs=clen, num_idxs_reg=clen, elem_size=128,
                                single_packet=False)
                            P = ppd.tile([128, maxc_sh], bft, tag="Pd", name="Pd")
                            P3 = P[:, :clen].rearrange("p (c d) -> p c d", d=128)
                            nc.vector.tensor_tensor(
                                P3,
                                iota_b[:].unsqueeze(1).broadcast_to([128, nch, 128]),
                                dlc[:, :nch]
                                    .unsqueeze(2).broadcast_to([128, nch, 128]),
                                mybir.AluOpType.is_equal)
                            for j in range(nch):
                                epos = cpos + j * 128
                                t = next(tt for tt, p0s, Ls in spans
                                         if p0s <= epos < p0s + Ls)
                                left[t] -= 1
                                nc.tensor.matmul(pst[t],
                                                 lhsT=P[:, j * 128:(j + 1) * 128],
                                                 rhs=gg[:, j * 128:(j + 1) * 128],
                                                 start=False,
                                                 stop=(left[t] == 0))

                        ap_calls = list(range(0, g["L"], CALL))
                        gmeta = sh_groups[gi]
                        dma_calls = []
                        for hh in (0, 1):
                            spans = gmeta[hh]
                            if not spans:
                                continue
                            gpos = spans[0][1]
                            gend = spans[-1][1] + spans[-1][2]
                            for cpos, clen in split_calls(gpos, gend - gpos, maxc_sh):
                                dma_calls.append((hh, cpos, clen, spans))
                        nA, nD = len(ap_calls), len(dma_calls)
                        ai = di = 0
                        while ai < nA or di < nD:
                            if di < nD and (ai >= nA or di * nA <= ai * nD):
                                emit_dma(dma_calls[di]); di += 1
                            else:
                                emit_ap(ap_calls[ai]); ai += 1
                        for t in tiles:
                            post(t, pst[t])

                tb = tbp.tile([128, NPC * 8], bft, tag="tb")
                tb3 = tb[:].rearrange("p (n d) -> p n d", d=8)
                nc.sync.dma_start(out=tb3, in_=tout[0][:, :, :])

                def post1(t, ps):
                    T2t = t2p.tile([128, 128], bft, tag="t2")
                    act_leaky(T2t[:], ps, deginv_pp, t)
                    T2p = t2p.tile([128, 128], bft, tag="t2p")
                    nc.vector.tensor_copy(
                        T2p[:].rearrange("p (j c) -> p j c", j=8, c=16),
                        T2t[:].rearrange("p (c j) -> p j c", c=16, j=8))
                    nc.sync.dma_start(out=slf[1][t * 128:t * 128 + nv(t), :],
                                      in_=T2p[:nv(t), :])
                    write_tin(1, t, T2t, itt1, psg8, trtag="psg")
                hybrid_pass(tb3, slf[0], tnm[0], brow["b1"], post1)
                allgather_ap(1)
                if DMA_SHARE_PM:
                    allgather_nm(1)

                tb2 = tbp.tile([128, NPC * 8], bft, tag="tb")
                tb23 = tb2[:].rearrange("p (n d) -> p n d", d=8)
                nc.sync.dma_start(out=tb23, in_=tout[1][:, :, :])

                def post2(t, ps):
                    o2 = t2p.tile([128, 128], bft, tag="s2o")
                    nc.vector.tensor_copy(o2[:], ps)
                    nc.sync.dma_start(out=s2d[t * 128:t * 128 + nv(t), :],
                                      in_=o2[:nv(t), :])
                hybrid_pass(tb23, slf[1], tnm[1], None, post2)

            # ---------- phase D2: dense2 + dense3 -> T3 ----------
            with tc.tile_pool(name="hp", bufs=6) as hp, \
                 tc.tile_pool(name="t3p", bufs=4) as t3p, \
                 tc.tile_pool(name="psD", bufs=2, space="PSUM") as psD, \
                 tc.tile_pool(name="trD", bufs=2, space="PSUM") as trD:
                for t in range(TPC):
                    s2l = hp.tile([128, 128], bft, tag="s2l")
                    r1 = min((t + 1) * 128, NPC)
                    if r1 - t * 128 < 128:
                        nc.vector.memzero(s2l[:])
                    nc.sync.dma_start(out=s2l[:r1 - t * 128, :],
                                      in_=s2d[t * 128:r1, :])
                    trs = trD.tile([128, 128], bft, tag="trs")
                    nc.tensor.matmul(trs[:], lhsT=s2l[:],
                                     rhs=ident_b[:], is_transpose=True)
                    s2t = hp.tile([128, 128], bft, tag="s2t")
                    nc.vector.tensor_copy(s2t[:], trs[:])
                    ps2 = psD.tile([128, 384], f32, tag="ps2")
                    nc.tensor.matmul(ps2[:], lhsT=sq_row[0:1, t * 128:(t + 1) * 128],
                                     rhs=brow["b2"][:], start=True, stop=False)
                    nc.tensor.matmul(ps2[:], lhsT=s2t[:],
                                     rhs=W2sb[0][:], start=False, stop=True)
                    h2 = hp.tile([128, 384], bft, tag="h2")
                    act_leaky(h2[:], ps2[:], dinv_pp, t)
                    trp = trD.tile([128, 384], bft, tag="tr")
                    for k in range(3):
                        nc.tensor.matmul(trp[:, k * 128:(k + 1) * 128],
                                         lhsT=h2[:, k * 128:(k + 1) * 128],
                                         rhs=ident_b[:], is_transpose=True)
                    h2t = hp.tile([128, 384], bft, tag="h2t")
                    nc.vector.tensor_copy(h2t[:], trp[:])
                    ps3 = psD.tile([128, 256], f32, tag="ps3")
                    for k in range(3):
                        nc.tensor.matmul(ps3[:], lhsT=h2t[:, k * 128:(k + 1) * 128],
                                         rhs=W3sb[k][:], start=(k == 0), stop=(k == 2))
                    T3t = t3p.tile([128, 256], bft, tag="t3")
                    nc.vector.tensor_scalar(T3t[:], ps3[:], dinv_pp[:, t:t + 1], None,
                                            mybir.AluOpType.mult)
                    nc.sync.dma_start(out=ag_in[2][t * 128:t * 128 + nv(t), :],
                                      in_=T3t[:nv(t), :])
            allgather(2)

            # ---------- late consts for full dma-gather passes ----------
            s4nm = cp.tile([128, 2 * NPAD], bft)       # S4 raw agg [node, 256]
            idxf_sb = cp.tile([128, TOTF // 16], dt.int16)
            nc.sync.dma_start(out=idxf_sb[:], in_=idxf_t[:, :])
            dlf_sb = cp.tile([128, TOTF // 128], bft)
            nc.sync.dma_start(out=dlf_sb[:], in_=dlf_t[:, :])

            # ---------- generic dma-gather aggregation pass (E / F1) ----------
            def agg_pass(pi, F, post, binit_bias=None):
                maxc = maxc128 if F == 128 else maxc256
                tab = table[pi]
                half0 = tab[0:HALF, :]
                half1 = tab[HALF:N, :]
                with tc.tile_pool(name=f"g{pi}", bufs=3) as gp, \
                     tc.tile_pool(name=f"pp{pi}", bufs=3) as pp, \
                     tc.tile_pool(name=f"sl{pi}", bufs=3) as slp, \
                     tc.tile_pool(name=f"agg{pi}", bufs=6, space="PSUM") as ap_:
                    for gmeta in full_cfg["groups"]:
                        tiles = sorted({t for hh in (0, 1) for t, _, _ in gmeta[hh]})
                        pst = {}
                        left = {t: sum(L for _, _, L in full_cfg["segs"][t]) // 128
                                for t in tiles}
                        for t in tiles:
                            ps = ap_.tile([128, F], f32, tag="agg", name="agg")
                            pst[t] = ps
                            if binit_bias is not None:
                                nc.tensor.matmul(ps[:],
                                                 lhsT=sq_row[0:1, t * 128:(t + 1) * 128],
                                                 rhs=binit_bias[:],
                                                 start=True, stop=False)
                            sl = slp.tile([128, F], bft, tag="sl", name="sl")
                            r1 = min((t + 1) * 128, NPC)
                            if r1 - t * 128 < 128:
                                nc.vector.memzero(sl[:])
                            nc.sync.dma_start(out=sl[:r1 - t * 128, :],
                                              in_=ag_in[pi][t * 128:r1, :])
                            nc.tensor.matmul(ps[:], lhsT=ident_b[:], rhs=sl[:],
                                             start=(binit_bias is None),
                                             stop=(left[t] == 0))
                        for hh in (0, 1):
                            src_ap = half0 if hh == 0 else half1
                            spans = gmeta[hh]
                            if not spans:
                                continue
                            gpos = spans[0][1]
                            gend = spans[-1][1] + spans[-1][2]
                            for cpos, clen in split_calls(gpos, gend - gpos, maxc):
                                nch = clen // 128
                                g = gp.tile([128, nch * F], bft, tag="g", name="g")
                                g3 = g[:].rearrange("p (c e) -> p c e", e=F)
                                nc.gpsimd.dma_gather(
                                    out_ap=g3, in_ap=src_ap,
                                    idxs_ap=idxf_sb[:, cpos // 16:(cpos + clen) // 16],
                                    num_idxs=clen, num_idxs_reg=clen, elem_size=F,
                                    single_packet=False)
                                P = pp.tile([128, clen], bft, tag="P", name="P")
                                P3 = P[:].rearrange("p (c d) -> p c d", d=128)
                                nc.vector.tensor_tensor(
                                    P3,
                                    iota_b[:].unsqueeze(1).broadcast_to([128, nch, 128]),
                                    dlf_sb[:, cpos // 128:(cpos + clen) // 128]
                                        .unsqueeze(2).broadcast_to([128, nch, 128]),
                                    mybir.AluOpType.is_equal)
                                for j in range(nch):
                                    epos = cpos + j * 128
                                    t = next(tt for tt, p0, L in spans
                                             if p0 <= epos < p0 + L)
                                    left[t] -= 1
                                    nc.tensor.matmul(pst[t][:],
                                                     lhsT=P[:, j * 128:(j + 1) * 128],
                                                     rhs=g[:, j * F:(j + 1) * F],
                                                     start=False,
                                                     stop=(left[t] == 0))
                        for t in tiles:
                            post(t, pst[t])

            # ---------- pass E: agg3 -> T4 ----------
            with tc.tile_pool(name="t4p", bufs=4) as t4p:
                def post_e(t, ps):
                    T4t = t4p.tile([128, 256], bft, tag="t4")
                    act_leaky(T4t[:], ps[:], deginv_pp, t)
                    nc.sync.dma_start(out=ag_in[3][t * 128:t * 128 + nv(t), :],
                                      in_=T4t[:nv(t), :])
                agg_pass(2, 256, post_e, binit_bias=brow["b3"])
            allgather(3)

            # ---------- pass F1: agg4 -> S4 (node-major) ----------
            def post_f1(t, ps):
                nc.vector.tensor_copy(s4nm[:, t * 256:(t + 1) * 256], ps[:])
            agg_pass(3, 256, post_f1)

            # ---------- phase F2: dense4 + dense5 -> out ----------
            with tc.tile_pool(name="hp4", bufs=6) as hp4, \
                 tc.tile_pool(name="op", bufs=4) as op, \
                 tc.tile_pool(name="psF", bufs=2, space="PSUM") as psF, \
                 tc.tile_pool(name="trF", bufs=2, space="PSUM") as trF:
                for t in range(TPC):
                    s4t = hp4.tile([128, 256], bft, tag="s4t")
                    for fk in range(2):
                        trs = trF.tile([128, 128], bft, tag="trs4")
                        nc.tensor.matmul(
                            trs[:],
                            lhsT=s4nm[:, t * 256 + fk * 128:t * 256 + (fk + 1) * 128],
                            rhs=ident_b[:], is_transpose=True)
                        nc.vector.tensor_copy(s4t[:, fk * 128:(fk + 1) * 128], trs[:])
                    ps4 = psF.tile([128, 384], f32, tag="ps4")
                    nc.tensor.matmul(ps4[:], lhsT=sq_row[0:1, t * 128:(t + 1) * 128],
                                     rhs=brow["b4"][:], start=True, stop=False)
                    for fk in range(2):
                        nc.tensor.matmul(ps4[:],
                                         lhsT=s4t[:, fk * 128:(fk + 1) * 128],
                                         rhs=W4sb[fk][:], start=False, stop=(fk == 1))
                    h4 = hp4.tile([128, 384], bft, tag="h4")
                    act_leaky(h4[:], ps4[:], dinv_pp, t)
                    trp = trF.tile([128, 384], bft, tag="tr4")
                    for k in range(3):
                        nc.tensor.matmul(trp[:, k * 128:(k + 1) * 128],
                                         lhsT=h4[:, k * 128:(k + 1) * 128],
                                         rhs=ident_b[:], is_transpose=True)
                    h4t = hp4.tile([128, 384], bft, tag="h4t")
                    nc.vector.tensor_copy(h4t[:], trp[:])
                    ps5 = psF.tile([128, 128], f32, tag="ps5")
                    nc.tensor.matmul(ps5[:], lhsT=ones_row[:], rhs=brow["bl"][:],
                                     start=True, stop=False)
                    for k in range(3):
                        nc.tensor.matmul(ps5[:], lhsT=h4t[:, k * 128:(k + 1) * 128],
                                         rhs=Wlsb[k][:], start=False, stop=(k == 2))
                    ot = op.tile([128, 128], f32, tag="o")
                    nc.scalar.activation(ot[:], ps5[:], mybir.ActivationFunctionType.Relu)
                    nc.sync.dma_start(out=out_t[t * 128:t * 128 + nv(t), :],
                                      in_=ot[:nv(t), :])

    nc.compile()
    return nc


def kernel(x, edge_index, W1, b1, W2, b2, W3, b3, W4, b4, Wl, bl,
           trace=False):
    x = np.asarray(x, dtype=np.float32)
    edge_index = np.asarray(edge_index)
    src = edge_index[0].astype(np.int64)
    dst = edge_index[1].astype(np.int64)
    deg = (np.bincount(dst, minlength=N) + 1).astype(np.float32)

    hsh = (src * 2654435761 + dst * 40503) % 1000
    mdma = hsh < DMA_SHARE_PM
    full_cfg, full_core = build_dma_layout(src, dst)
    share_cfg, share_core = build_dma_layout(src[mdma], dst[mdma])
    ap_groups, ap_core, TOTP, NCOL = build_ap_layout(src[~mdma], dst[~mdma])
    node_core = build_node_data(x, deg)

    piarr = np.array([8 * (m % 16) + m // 16 for m in range(128)])
    pi2 = piarr[piarr]
    W2p = np.asarray(W2)[pi2, :]
    b1p = np.asarray(b1)[piarr]
    wshared = dict(
        W1=np.asarray(W1).astype(bf16), W2=W2p.astype(bf16),
        W3=np.asarray(W3).astype(bf16), W4=np.asarray(W4).astype(bf16),
        Wl=np.asarray(Wl).astype(bf16),
        b1=b1p.reshape(1, -1).astype(bf16),
        b2=np.asarray(b2).reshape(1, -1).astype(bf16),
        b3=np.asarray(b3).reshape(1, -1).astype(bf16),
        b4=np.asarray(b4).reshape(1, -1).astype(bf16),
        bl=np.asarray(bl).reshape(1, -1).astype(bf16),
    )

    nc = build_program(full_cfg, share_cfg, ap_groups, TOTP, NCOL)
    in_maps = []
    for c in range(NC):
        m = {"xtt": node_core[c]["xtt"],
             "idxf": full_core[c]["idx"], "dlf": full_core[c]["dl"],
             "idxs": share_core[c]["idx"], "dls": share_core[c]["dl"],
             "deg_pp": node_core[c]["deg_pp"], "deg_row": node_core[c]["deg_row"],
             "idx8": ap_core[c]["idx8"], "dl8": ap_core[c]["dl8"]}
        m.update(wshared)
        in_maps.append(m)
    res = run_bass_kernel_spmd(nc, in_maps, core_ids=list(range(NC)),
                               trace=trace)
    out = np.concatenate([res.results[c]["out"] for c in range(NC)], axis=0)
    kernel.last_exec_time_ns = res.exec_time_ns
    kernel.last_results = res
    return out
